# revision 1
# baseline (speedup 1.0000x reference)
"""Trainium2 Bass kernel for nn_Block_local (dual global/banded-local attention block).

Sharding: pure data-parallel — one batch element per NeuronCore (B=8, 8 cores).
Per-core dataflow is feature-major (activations stored transposed, [C, N]) so every
linear layer is a single chain of PE matmuls with naturally-stored weights.
All matmuls run in float32r (TF32-like, full PE rate at free-dim >= 256).
"""
import os
import numpy as np

import concourse.bass as bass
import concourse.bacc as bacc
import concourse.mybir as mybir
import concourse.tile as tile
from concourse.bass_utils import run_bass_kernel_spmd
from concourse.masks import make_identity
from concourse import bass_isa
from contextlib import ExitStack

F32 = mybir.dt.float32
F32R = mybir.dt.float32r
AF = mybir.ActivationFunctionType
ALU = mybir.AluOpType
AX = mybir.AxisListType

B, N, C = 8, 1024, 768
GD = 384          # global (and local) feature dim
H, D = 6, 64      # heads, head dim
SCALE = D ** -0.5
HID = 3072
EPS = 1e-6
NH = 2            # token n-halves of 512
NHW = N // NH     # 512
MC = N // 128     # 8 token chunks
CC = C // 128     # 6 feature chunks
GC = GD // 128    # 3 feature chunks per branch
JC = HID // 128   # 24 hidden chunks


def f32(ap):
    return ap.bitcast(F32)


def _build(flags):
    nc = bacc.Bacc("TRN2", target_bir_lowering=False, debug=False)

    x_d = nc.dram_tensor("x", (N, C), F32, kind="ExternalInput")
    ln1_g = nc.dram_tensor("ln1_g", (GD,), F32, kind="ExternalInput")
    ln1_b = nc.dram_tensor("ln1_b", (GD,), F32, kind="ExternalInput")
    ln1l_g = nc.dram_tensor("ln1l_g", (GD,), F32, kind="ExternalInput")
    ln1l_b = nc.dram_tensor("ln1l_b", (GD,), F32, kind="ExternalInput")
    g_qkv_d = nc.dram_tensor("g_qkv_w", (GD, 3 * GD), F32, kind="ExternalInput")
    g_proj_d = nc.dram_tensor("g_proj_w", (GD, GD), F32, kind="ExternalInput")
    g_projb_d = nc.dram_tensor("g_proj_b", (GD,), F32, kind="ExternalInput")
    l_qkv_d = nc.dram_tensor("l_qkv_w", (GD, 3 * GD), F32, kind="ExternalInput")
    l_proj_d = nc.dram_tensor("l_proj_w", (GD, GD), F32, kind="ExternalInput")
    l_projb_d = nc.dram_tensor("l_proj_b", (GD,), F32, kind="ExternalInput")
    ln2_g = nc.dram_tensor("ln2_g", (C,), F32, kind="ExternalInput")
    ln2_b = nc.dram_tensor("ln2_b", (C,), F32, kind="ExternalInput")
    fc1_d = nc.dram_tensor("fc1_w", (C, HID), F32, kind="ExternalInput")
    fc1b_d = nc.dram_tensor("fc1_b", (HID,), F32, kind="ExternalInput")
    fc2_d = nc.dram_tensor("fc2_w", (HID, C), F32, kind="ExternalInput")
    fc2b_d = nc.dram_tensor("fc2_b", (C,), F32, kind="ExternalInput")
    out_d = nc.dram_tensor("out", (N, C), F32, kind="ExternalOutput")

    with tile.TileContext(nc) as tc, ExitStack() as top:
        consts = top.enter_context(tc.tile_pool(name="consts", bufs=1))
        core = top.enter_context(tc.tile_pool(name="core", bufs=1))

        ident = consts.tile([128, 128], F32, tag="ident")
        make_identity(nc, ident)
        ones = consts.tile([128, 128], F32, tag="ones")
        nc.vector.memset(ones, 1.0)
        ones_r = consts.tile([128, 128], F32R, tag="ones_r")
        nc.vector.tensor_copy(ones_r, ones)
        eps_t = consts.tile([128, 1], F32, tag="eps")
        nc.vector.memset(eps_t, EPS)
        zeros_t = consts.tile([128, 512], F32, tag="zeros")
        nc.vector.memset(zeros_t, 0.0)

        def load_vec(dram, n_elems, tag):
            # [n] -> per-partition layout [128, n//128]
            t = consts.tile([128, n_elems // 128], F32, tag=tag)
            nc.sync.dma_start(t, dram.rearrange("(c p) -> p c", p=128))
            return t

        g1g = load_vec(ln1_g, GD, "g1g") if flags["gb1g"] else None
        g1b = load_vec(ln1_b, GD, "g1b") if flags["gb1g"] else None
        l1g = load_vec(ln1l_g, GD, "l1g") if flags["gb1l"] else None
        l1b = load_vec(ln1l_b, GD, "l1b") if flags["gb1l"] else None
        g2g = load_vec(ln2_g, C, "g2g") if flags["gb2"] else None
        g2b = load_vec(ln2_b, C, "g2b") if flags["gb2"] else None
        gpb = load_vec(g_projb_d, GD, "gpb") if flags["bias_gproj"] else None
        lpb = load_vec(l_projb_d, GD, "lpb") if flags["bias_lproj"] else None
        fc1b = load_vec(fc1b_d, HID, "fc1b") if flags["bias_fc1"] else None
        fc2b = load_vec(fc2b_d, C, "fc2b") if flags["bias_fc2"] else None

        # resident full-block activations (fp32r, rounded on write)
        xT = core.tile([128, CC, N], F32R, tag="xT")       # x^T then x1^T (residual updated in place)


        # ---------------- feature-major LayerNorm helper ----------------
        def ln_feat(src, lo, hi, dst, dlo, gv, bv, sq_p, st_p, bc_p):
            """dst[:, dlo + (c-lo), :] = LN(src rows [lo*128, hi*128)) along features."""
            nch = hi - lo
            inv = 1.0 / (nch * 128)
            for nh in range(NH):
                ns = slice(nh * NHW, (nh + 1) * NHW)
                st = st_p.tile([1, 2 * NHW], F32, tag="stat")
                for i, c in enumerate(range(lo, hi)):
                    nc.tensor.matmul(st[:, 0:NHW], ones_r[:, 0:1], src[:, c, ns],
                                     start=(i == 0), stop=(i == nch - 1))
                for i, c in enumerate(range(lo, hi)):
                    sq = sq_p.tile([128, NHW], F32R, tag="sq")
                    nc.vector.tensor_tensor(sq, f32(src[:, c, ns]), f32(src[:, c, ns]), ALU.mult)
                    nc.tensor.matmul(st[:, NHW:2 * NHW], ones_r[:, 0:1], sq,
                                     start=(i == 0), stop=(i == nch - 1))
                mean = sq_p.tile([1, NHW], F32R, tag="mean")
                nc.vector.tensor_scalar_mul(mean, st[:, 0:NHW], inv)
                e2 = sq_p.tile([1, NHW], F32, tag="e2")
                nc.vector.tensor_scalar_mul(e2, st[:, NHW:2 * NHW], inv)
                var = sq_p.tile([1, NHW], F32, tag="var")
                nc.vector.tensor_tensor(var, f32(mean), f32(mean), ALU.mult)
                nc.vector.tensor_tensor(var, e2, var, ALU.subtract)
                sr = sq_p.tile([1, NHW], F32, tag="sr")
                nc.scalar.activation(sr, var, AF.Sqrt, bias=eps_t[0:1, :], scale=1.0)
                rstd = sq_p.tile([1, NHW], F32R, tag="rstd")
                with nc.allow_low_precision(reason="f32r rounding for matmul operand"):
                    nc.vector.reciprocal(rstd, sr)
                mb = bc_p.tile([128, NHW], F32, tag="mb")
                nc.tensor.matmul(mb, ones_r[0:1, :], mean, start=True, stop=True)
                rb = bc_p.tile([128, NHW], F32, tag="rb")
                nc.tensor.matmul(rb, ones_r[0:1, :], rstd, start=True, stop=True)
                for c in range(lo, hi):
                    dslice = dst[:, dlo + (c - lo), ns]
                    tmp = sq_p.tile([128, NHW], F32, tag="xm")
                    nc.vector.tensor_tensor(tmp, f32(src[:, c, ns]), mb, ALU.subtract)
                    if gv is not None:
                        nc.vector.tensor_tensor(tmp, tmp, rb, ALU.mult)
                        nc.vector.tensor_scalar(dslice, tmp, gv[:, c - lo:c - lo + 1],
                                                bv[:, c - lo:c - lo + 1], ALU.mult, ALU.add)
                    else:
                        nc.vector.tensor_tensor(dslice, tmp, rb, ALU.mult)

        # ---------------- phase 0: load x, transpose to feature-major ----------------
        with tc.tile_pool(name="xtok", bufs=4) as xtok_p, \
             tc.tile_pool(name="ps_tr0", bufs=6, space="PSUM") as ps_tr0:
            for m in range(MC):
                xt = xtok_p.tile([128, C], F32, tag="xt")
                nc.sync.dma_start(xt, x_d[m * 128:(m + 1) * 128, :])
                for c in range(CC):
                    ps = ps_tr0.tile([128, 128], F32, tag="tr")
                    nc.tensor.transpose(ps, xt[:, c * 128:(c + 1) * 128], ident)
                    if (c + m) % 2 == 0:
                        nc.vector.tensor_copy(xT[:, c, m * 128:(m + 1) * 128], ps)
                    else:
                        nc.scalar.copy(xT[:, c, m * 128:(m + 1) * 128], ps)

        # ---------------- phase 1: LN1 (both halves) ----------------
        with tc.tile_pool(name="ln1out", bufs=1) as ln1_p, \
             tc.tile_pool(name="qkvl", bufs=1) as qkvl_p:
            xgln = ln1_p.tile([128, GC, N], F32R, tag="xgln")
            xlln = ln1_p.tile([128, GC, N], F32R, tag="xlln")
            with tc.tile_pool(name="sq1", bufs=4) as sq_p, \
                 tc.tile_pool(name="st1", bufs=2, space="PSUM") as st_p, \
                 tc.tile_pool(name="bc1", bufs=2, space="PSUM") as bc_p:
                ln_feat(xT, 0, GC, xgln, 0, g1g, g1b, sq_p, st_p, bc_p)
                ln_feat(xT, GC, CC, xlln, 0, l1g, l1b, sq_p, st_p, bc_p)

            # ---------------- phase 2: global attention ----------------
            with tc.tile_pool(name="gattn", bufs=1) as ga_p, \
                 tc.tile_pool(name="wstage", bufs=1) as wst_p, \
                 tc.tile_pool(name="esb", bufs=3) as e_p, \
                 tc.tile_pool(name="small", bufs=3) as sm_p, \
                 tc.tile_pool(name="pq", bufs=2, space="PSUM") as pq_p, \
                 tc.tile_pool(name="psc", bufs=2, space="PSUM") as ps_p, \
                 tc.tile_pool(name="po", bufs=2, space="PSUM") as po_p:

                # weights: stage fp32 then round to f32r on gpsimd
                def stage_round(dst_shape, tag, fill):
                    st = wst_p.tile(dst_shape, F32, tag="wstage")
                    fill(st)
                    dst = ga_p.tile(dst_shape, F32R, tag=tag)
                    nc.gpsimd.tensor_copy(out=dst, in_=st)
                    return dst

                gqkv_v = g_qkv_d.rearrange("(kc p) c -> p kc c", p=128)
                gqk_r = stage_round([128, GC, 2 * GD], "gqk",
                                    lambda t: nc.sync.dma_start(t, gqkv_v[:, :, 0:2 * GD]))

                def fill_vpad(t):
                    nc.vector.memset(t, 0.0)
                    tv = t.rearrange("p kc (h e) -> p kc h e", e=D + 1)
                    src = gqkv_v[:, :, 2 * GD:3 * GD].rearrange("p kc (h d) -> p kc h d", d=D)
                    for kc in range(GC):
                        nc.sync.dma_start(tv[:, kc, :, 0:D], src[:, kc])
                wvp_r = stage_round([128, GC, H * (D + 1)], "wvp", fill_vpad)
                gproj_r = stage_round([128, GC, GD], "gproj",
                                      lambda t: nc.sync.dma_start(
                                          t, g_proj_d.rearrange("(kc p) c -> p kc c", p=128)))
                lqkv_r = stage_round([128, GC, 3 * GD], "lqkv",
                                     lambda t: nc.sync.dma_start(
                                         t, l_qkv_d.rearrange("(kc p) c -> p kc c", p=128)))
                ql = qkvl_p.tile([128, MC, GD], F32, tag="ql")
                kl = qkvl_p.tile([128, MC, GD], F32, tag="kl")
                vl = qkvl_p.tile([128, MC, GD], F32, tag="vl")
                lq_groups = [(m, pi) for m in range(MC) for pi in range(3)]

                def emit_lqkv(n):
                    # local qkv matmuls dripped into the global-attention PE
                    # stream: they fill gaps where scores wait on ACT exp.
                    for _ in range(n):
                        if not lq_groups:
                            return
                        m, pi = lq_groups.pop(0)
                        dst = (ql, kl, vl)[pi]
                        ps_l = pq_p.tile([128, NHW], F32, tag="pq", name="lqkv_ps")
                        psd = ps_l[:, 0:GD]
                        for kc in range(GC):
                            nc.tensor.matmul(psd, xlln[:, kc, m * 128:(m + 1) * 128],
                                             lqkv_r[:, kc, pi * GD:(pi + 1) * GD],
                                             start=(kc == 0), stop=(kc == GC - 1))
                        nc.vector.tensor_copy(dst[:, m, :], psd)

                qT = ga_p.tile([128, GC, N], F32R, tag="qT")
                kT = ga_p.tile([128, GC, N], F32R, tag="kT")
                vpad = ga_p.tile([128, MC, H * (D + 1)], F32R, tag="vpad")
                oT = ga_p.tile([128, GC, N], F32R, tag="oT")

                # Q^T, K^T: [2GD, n] = gqk.T @ xgln
                for mo in range(2 * GC):
                    dst = qT if mo < GC else kT
                    dc = mo % GC
                    for nh in range(NH):
                        ns = slice(nh * NHW, (nh + 1) * NHW)
                        ps = pq_p.tile([128, NHW], F32, tag="pq")
                        for kc in range(GC):
                            nc.tensor.matmul(ps, gqk_r[:, kc, mo * 128:(mo + 1) * 128],
                                             xgln[:, kc, ns], start=(kc == 0), stop=(kc == GC - 1))
                        if (mo + nh) % 2 == 0:
                            nc.vector.tensor_copy(dst[:, dc, ns], ps)
                        else:
                            nc.scalar.copy(dst[:, dc, ns], ps)

                # V (token-major, head-padded with ones column)
                for m in range(MC):
                    ps = pq_p.tile([128, NHW], F32, tag="pq")
                    psv = ps[:, 0:H * (D + 1)]
                    for kc in range(GC):
                        nc.tensor.matmul(psv, xgln[:, kc, m * 128:(m + 1) * 128],
                                         wvp_r[:, kc, :], start=(kc == 0), stop=(kc == GC - 1))
                    if m % 2 == 0:
                        nc.vector.tensor_copy(vpad[:, m, :], psv)
                    else:
                        nc.scalar.copy(vpad[:, m, :], psv)
                    nc.vector.tensor_copy(
                        vpad[:, m].rearrange("p (h e) -> p h e", e=D + 1)[:, :, D],
                        ones[:, 0:H])

                # scores^T -> exp -> O^T accumulation, per head / n-half.
                # m-chunks in pairs: two S^T matmuls fill the two banks of one
                # [128, 1024] PSUM tile; a single ACT exp op covers both,
                # halving ACT per-op overhead (the phase limiter).
                for h in range(H):
                    hc, hp = h // 2, (h % 2) * 64
                    for nh in range(NH):
                        ns = slice(nh * NHW, (nh + 1) * NHW)
                        po = po_p.tile([D + 1, NHW], F32, tag="po")
                        for mp in range(MC // 2):
                            ps = ps_p.tile([128, 2 * NHW], F32, tag="ps")
                            for half in range(2):
                                m = 2 * mp + half
                                nc.tensor.matmul(ps[:, half * NHW:(half + 1) * NHW],
                                                 kT[hp:hp + 64, hc, m * 128:(m + 1) * 128],
                                                 qT[hp:hp + 64, hc, ns], start=True, stop=True)
                            e_sb = e_p.tile([128, 2 * NHW], F32R, tag="e")
                            nc.scalar.activation(e_sb, ps, AF.Exp, scale=SCALE)
                            for half in range(2):
                                m = 2 * mp + half
                                nc.tensor.matmul(po, vpad[:, m, h * (D + 1):(h + 1) * (D + 1)],
                                                 e_sb[:, half * NHW:(half + 1) * NHW],
                                                 start=(m == 0), stop=(m == MC - 1))
                        rcp = sm_p.tile([1, NHW], F32R, tag="rcp")
                        with nc.allow_low_precision(reason="f32r rounding for matmul operand"):
                            nc.vector.reciprocal(rcp, po[D:D + 1, :])
                        pb = pq_p.tile([128, NHW], F32, tag="pq", name="pbbc")[0:64, :]
                        nc.tensor.matmul(pb, ones_r[0:1, 0:64], rcp, start=True, stop=True)
                        pb_sb = sm_p.tile([64, NHW], F32, tag="pbsb")
                        nc.vector.tensor_copy(pb_sb, pb)
                        nc.vector.tensor_tensor(oT[hp:hp + 64, hc, ns], po[0:D, :], pb_sb, ALU.mult)
                    emit_lqkv(4)
                emit_lqkv(len(lq_groups))

                # proj + residual into xT rows [0, GD)
                for mo in range(GC):
                    for nh in range(NH):
                        ns = slice(nh * NHW, (nh + 1) * NHW)
                        ps = pq_p.tile([128, NHW], F32, tag="pq")
                        for kc in range(GC):
                            nc.tensor.matmul(ps, gproj_r[:, kc, mo * 128:(mo + 1) * 128],
                                             oT[:, kc, ns], start=(kc == 0), stop=(kc == GC - 1))
                        if gpb is not None:
                            nc.scalar.activation(ps, ps, AF.Identity,
                                                 bias=gpb[:, mo:mo + 1], scale=1.0)
                        nc.vector.tensor_tensor(xT[:, mo, ns], f32(xT[:, mo, ns]), ps, ALU.add)

            # ---------------- phase 3: local (banded) attention ----------------
            with tc.tile_pool(name="lattn", bufs=1) as la_p, \
                 tc.tile_pool(name="wstage2", bufs=1) as wst2_p, \
                 tc.tile_pool(name="lwork", bufs=4) as lw_p, \
                 tc.tile_pool(name="pq2", bufs=4, space="PSUM") as pq2_p, \
                 tc.tile_pool(name="ptr2", bufs=4, space="PSUM") as pt2_p:

                st2 = wst2_p.tile([128, GC, GD], F32, tag="wstage2b")
                nc.sync.dma_start(st2, l_proj_d.rearrange("(kc p) c -> p kc c", p=128))
                lproj_r = la_p.tile([128, GC, GD], F32R, tag="lproj")
                nc.gpsimd.tensor_copy(out=lproj_r, in_=st2)

                # token-shifted copies (prev/next), zero at sequence edges
                km = la_p.tile([128, MC, GD], F32, tag="km")
                kp = la_p.tile([128, MC, GD], F32, tag="kp")
                vm = la_p.tile([128, MC, GD], F32, tag="vm")
                vp = la_p.tile([128, MC, GD], F32, tag="vp")
                for src, dst, d in ((kl, km, -1), (vl, vm, -1), (kl, kp, 1), (vl, vp, 1)):
                    if d == -1:
                        nc.sync.dma_start(dst[1:128, :, :], src[0:127, :, :])
                        nc.sync.dma_start(dst[0:1, 1:MC, :], src[127:128, 0:MC - 1, :])
                        # token 0 has no predecessor: zero the row (keeps 0*w finite)
                        nc.sync.dma_start(dst[0:1, 0:1, :], zeros_t[0:1, 0:GD])
                    else:
                        nc.sync.dma_start(dst[0:127, :, :], src[1:128, :, :])
                        nc.sync.dma_start(dst[127:128, 0:MC - 1, :], src[0:1, 1:MC, :])
                        # token N-1 has no successor: zero the row
                        nc.sync.dma_start(dst[127:128, MC - 1:MC, :], zeros_t[0:1, 0:GD])

                ol = la_p.tile([128, MC, GD], F32, tag="ol")
                for m in range(MC):
                    ed = lw_p.tile([128, H, 3], F32, tag="ed")
                    for di, kk in enumerate((km, kl, kp)):
                        prod = lw_p.tile([128, GD], F32, tag="prod")
                        nc.vector.tensor_tensor(prod, ql[:, m, :], kk[:, m, :], ALU.mult)
                        nc.vector.reduce_sum(ed[:, :, di],
                                             prod.rearrange("p (h d) -> p h d", d=D), axis=AX.X)
                    ee = lw_p.tile([128, H, 3], F32, tag="ee")
                    nc.scalar.activation(ee, ed, AF.Exp, scale=SCALE)
                    if m == 0:
                        nc.vector.memset(ee[0:1, :, 0], 0.0)
                    if m == MC - 1:
                        nc.sync.dma_start(ee[127:128, :, 2], zeros_t[0:1, 0:H])
                    ssum = lw_p.tile([128, H], F32, tag="ssum")
                    nc.vector.reduce_sum(ssum, ee, axis=AX.X)
                    rr = lw_p.tile([128, H], F32, tag="rr")
                    nc.vector.reciprocal(rr, ssum)
                    ov = ol[:, m].rearrange("p (h d) -> p h d", d=D)
                    for di, vv in enumerate((vm, vl, vp)):
                        aw = lw_p.tile([128, H], F32, tag=f"aw{di}")
                        nc.vector.tensor_tensor(aw, ee[:, :, di], rr, ALU.mult)
                        awb = aw[:, :, None].to_broadcast((128, H, D))
                        vvv = vv[:, m].rearrange("p (h d) -> p h d", d=D)
                        if di == 0:
                            nc.vector.tensor_tensor(ov, vvv, awb, ALU.mult)
                        else:
                            t = lw_p.tile([128, H, D], F32, tag="avt")
                            nc.vector.tensor_tensor(t, vvv, awb, ALU.mult)
                            nc.vector.tensor_tensor(ov, ov, t, ALU.add)

                # transpose O_l to feature-major
                oTl = la_p.tile([128, GC, N], F32R, tag="oTl")
                for m in range(MC):
                    for c in range(GC):
                        ps = pt2_p.tile([128, 128], F32, tag="tr2")
                        nc.tensor.transpose(ps, ol[:, m, c * 128:(c + 1) * 128], ident)
                        if (m + c) % 2 == 0:
                            nc.vector.tensor_copy(oTl[:, c, m * 128:(m + 1) * 128], ps)
                        else:
                            nc.scalar.copy(oTl[:, c, m * 128:(m + 1) * 128], ps)

                # local proj + residual into xT rows [GD, C)
                for mo in range(GC):
                    for nh in range(NH):
                        ns = slice(nh * NHW, (nh + 1) * NHW)
                        ps = pq2_p.tile([128, NHW], F32, tag="pq2")
                        for kc in range(GC):
                            nc.tensor.matmul(ps, lproj_r[:, kc, mo * 128:(mo + 1) * 128],
                                             oTl[:, kc, ns], start=(kc == 0), stop=(kc == GC - 1))
                        if lpb is not None:
                            nc.scalar.activation(ps, ps, AF.Identity,
                                                 bias=lpb[:, mo:mo + 1], scale=1.0)
                        nc.vector.tensor_tensor(xT[:, GC + mo, ns], f32(xT[:, GC + mo, ns]),
                                                ps, ALU.add)

        # ---------------- phase 4: LN2 ----------------
        tail = top.enter_context(tc.tile_pool(name="tail", bufs=1))
        hT = tail.tile([128, CC, N], F32R, tag="hT")
        outT = tail.tile([128, CC, N], F32, tag="outT")
        if flags["gb2"]:
            with tc.tile_pool(name="sq2", bufs=4) as sq_p, \
                 tc.tile_pool(name="st2p", bufs=2, space="PSUM") as st_p, \
                 tc.tile_pool(name="bc2", bufs=2, space="PSUM") as bc_p:
                ln_feat(xT, 0, CC, hT, 0, g2g, g2b, sq_p, st_p, bc_p)

        # ---------------- phase 5: MLP (fc1 resident, fc2 streamed) ----------------
        with tc.tile_pool(name="mlp", bufs=1) as mlp_p, \
             tc.tile_pool(name="w1stage", bufs=2) as w1s_p, \
             tc.tile_pool(name="w2stage", bufs=3) as w2s_p, \
             tc.tile_pool(name="w2r", bufs=3) as w2r_p, \
             tc.tile_pool(name="gl", bufs=2) as gl_p, \
             tc.tile_pool(name="lnw", bufs=1) as lnw_p, \
             tc.tile_pool(name="pz", bufs=1, space="PSUM") as pz_p, \
             tc.tile_pool(name="pm", bufs=2, space="PSUM") as pm_p:
            fc1_r = mlp_p.tile([128, CC, HID], F32R, tag="fc1")
            fc1_v = fc1_d.rearrange("(kc p) h -> p kc h", p=128)
            for kc in range(CC):
                for hh in range(2):
                    hs = slice(hh * (HID // 2), (hh + 1) * (HID // 2))
                    st = w1s_p.tile([128, HID // 2], F32, tag="w1stage")
                    nc.sync.dma_start(st, fc1_v[:, kc, hs])
                    nc.gpsimd.tensor_copy(out=fc1_r[:, kc, hs], in_=st)

            def ln2_allreduce(nh):
                # PSUM-free LN2 (stats via gpsimd all-reduce) so it can live
                # inside the MLP scope: half nh=1's LN2 hides under nh=0's
                # matmul stream.
                ns = slice(nh * NHW, (nh + 1) * NHW)
                inv = 1.0 / C
                xs = lnw_p.tile([128, NHW], F32, tag="xs")
                nc.vector.tensor_tensor(xs, f32(xT[:, 0, ns]), f32(xT[:, 1, ns]), ALU.add)
                for c in range(2, CC):
                    nc.vector.tensor_tensor(xs, xs, f32(xT[:, c, ns]), ALU.add)
                sqs = lnw_p.tile([128, NHW], F32, tag="sqs")
                nc.vector.tensor_tensor(sqs, f32(xT[:, 0, ns]), f32(xT[:, 0, ns]), ALU.mult)
                for c in range(1, CC):
                    tmp = lnw_p.tile([128, NHW], F32, tag="sqtmp")
                    nc.vector.tensor_tensor(tmp, f32(xT[:, c, ns]), f32(xT[:, c, ns]), ALU.mult)
                    nc.vector.tensor_tensor(sqs, sqs, tmp, ALU.add)
                xs_b = lnw_p.tile([128, NHW], F32, tag="xsb")
                nc.gpsimd.partition_all_reduce(xs_b, xs, channels=128,
                                               reduce_op=bass_isa.ReduceOp.add)
                sq_b = lnw_p.tile([128, NHW], F32, tag="sqb")
                nc.gpsimd.partition_all_reduce(sq_b, sqs, channels=128,
                                               reduce_op=bass_isa.ReduceOp.add)
                mean_b = lnw_p.tile([128, NHW], F32, tag="meanb")
                nc.vector.tensor_scalar_mul(mean_b, xs_b, inv)
                var_b = lnw_p.tile([128, NHW], F32, tag="varb")
                nc.vector.tensor_tensor(var_b, mean_b, mean_b, ALU.mult)
                nc.vector.tensor_scalar_mul(sq_b, sq_b, inv)
                nc.vector.tensor_tensor(var_b, sq_b, var_b, ALU.subtract)
                nc.scalar.activation(var_b, var_b, AF.Sqrt, bias=eps_t, scale=1.0)
                rstd_b = lnw_p.tile([128, NHW], F32, tag="rstdb")
                nc.vector.reciprocal(rstd_b, var_b)
                for c in range(CC):
                    tmp2 = lnw_p.tile([128, NHW], F32, tag="xm2")
                    nc.vector.tensor_tensor(tmp2, f32(xT[:, c, ns]), mean_b, ALU.subtract)
                    nc.vector.tensor_tensor(hT[:, c, ns], tmp2, rstd_b, ALU.mult)

            for nh in range(NH):
                if not flags["gb2"]:
                    ln2_allreduce(nh)
                ns = slice(nh * NHW, (nh + 1) * NHW)
                zps = [pz_p.tile([128, NHW], F32, tag=f"z{mo}", name=f"z{mo}") for mo in range(CC)]
                # fc2(j) emitted one step behind fc1(j+1): PE streams fc1(j+1)
                # while ACT runs gelu(j), so fc2 never stalls on gelu.
                pend = None
                for j in range(JC):
                    pm = pm_p.tile([128, NHW], F32, tag="pm")
                    for kc in range(CC):
                        nc.tensor.matmul(pm, fc1_r[:, kc, j * 128:(j + 1) * 128],
                                         hT[:, kc, ns], start=(kc == 0), stop=(kc == CC - 1))
                    gl = gl_p.tile([128, NHW], F32R, tag="gl")
                    gbias = fc1b[:, j:j + 1] if fc1b is not None else 0.0
                    nc.scalar.activation(gl, pm, AF.Gelu, bias=gbias, scale=1.0)
                    w2s = w2s_p.tile([128, C], F32, tag="w2stage")
                    nc.sync.dma_start(w2s, fc2_d[j * 128:(j + 1) * 128, :])
                    w2r = w2r_p.tile([128, C], F32R, tag="w2r")
                    nc.gpsimd.tensor_copy(out=w2r, in_=w2s)
                    if pend is not None:
                        pg, pw, pj = pend
                        for mo in range(CC):
                            nc.tensor.matmul(zps[mo], pw[:, mo * 128:(mo + 1) * 128], pg,
                                             start=(pj == 0), stop=(pj == JC - 1))
                    pend = (gl, w2r, j)
                pg, pw, pj = pend
                for mo in range(CC):
                    nc.tensor.matmul(zps[mo], pw[:, mo * 128:(mo + 1) * 128], pg,
                                     start=(pj == 0), stop=(pj == JC - 1))
                for mo in range(CC):
                    if fc2b is not None:
                        nc.scalar.activation(zps[mo], zps[mo], AF.Identity,
                                             bias=fc2b[:, mo:mo + 1], scale=1.0)
                    nc.vector.tensor_tensor(outT[:, mo, ns], f32(xT[:, mo, ns]), zps[mo], ALU.add)

        # ---------------- phase 6: transpose back + store ----------------
        with tc.tile_pool(name="otok", bufs=3) as otok_p, \
             tc.tile_pool(name="ps_tr3", bufs=4, space="PSUM") as ps_tr3:
            for m in range(MC):
                ot = otok_p.tile([128, C], F32, tag="ot")
                for c in range(CC):
                    ps = ps_tr3.tile([128, 128], F32, tag="tr3")
                    nc.tensor.transpose(ps, outT[:, c, m * 128:(m + 1) * 128], ident)
                    if (c + m) % 2 == 0:
                        nc.vector.tensor_copy(ot[:, c * 128:(c + 1) * 128], ps)
                    else:
                        nc.scalar.copy(ot[:, c * 128:(c + 1) * 128], ps)
                nc.sync.dma_start(out_d[m * 128:(m + 1) * 128, :], ot)

    nc.compile()
    return nc


_NC_CACHE = {}


def kernel(**inputs):
    inp = {k: np.ascontiguousarray(np.asarray(v), dtype=np.float32) for k, v in inputs.items()}
    flags = {
        "gb1g": not (np.all(inp["ln1_g"] == 1.0) and np.all(inp["ln1_b"] == 0.0)),
        "gb1l": not (np.all(inp["ln1l_g"] == 1.0) and np.all(inp["ln1l_b"] == 0.0)),
        "gb2": not (np.all(inp["ln2_g"] == 1.0) and np.all(inp["ln2_b"] == 0.0)),
        "bias_gproj": bool(np.any(inp["g_proj_b"] != 0.0)),
        "bias_lproj": bool(np.any(inp["l_proj_b"] != 0.0)),
        "bias_fc1": bool(np.any(inp["fc1_b"] != 0.0)),
        "bias_fc2": bool(np.any(inp["fc2_b"] != 0.0)),
    }
    key = tuple(sorted(flags.items()))
    nc = _NC_CACHE.get(key)
    if nc is None:
        nc = _build(flags)
        _NC_CACHE[key] = nc
    x = inp["x"]
    weights = {k: v for k, v in inp.items() if k != "x"}
    in_maps = [dict(weights, x=np.ascontiguousarray(x[b])) for b in range(B)]
    trace = os.environ.get("BASS_KERNEL_TRACE", "") == "1"
    res = run_bass_kernel_spmd(nc, in_maps, core_ids=list(range(B)),
                               trace=trace, trace_cores=[0] if trace else None)
    if trace:
        print(f"HW exec time: {res.exec_time_ns} ns")
        if res.instructions_and_trace:
            print("trace path:", res.instructions_and_trace[1])
    return np.stack([res.results[b]["out"] for b in range(B)]).astype(np.float32)



# revision 14
# speedup vs baseline: 1.3532x; 1.3532x over previous
"""Trainium2 Bass kernel for nn_Block_local (dual global/banded-local attention block).

Sharding: data-parallel, one batch element per NeuronCore (B=8, 8 cores).
Feature-major activations ([C,N]); fp8e4 DoubleRow matmuls for all
weight-contractions (weights quantized host-side, fc1/fc2 split hi+lo fp8),
bf16 scores, fp8 softmax/activation intermediates, feature-major banded local
attention (shifts are free-axis slices; no shift DMAs, no local transposes).
"""
import os
import numpy as np
import ml_dtypes

import concourse.bass as bass
import concourse.bacc as bacc
import concourse.mybir as mybir
import concourse.tile as tile
from concourse.bass_utils import run_bass_kernel_spmd
from concourse.masks import make_identity
from contextlib import ExitStack

F32 = mybir.dt.float32
F32R = mybir.dt.float32r
BF16 = mybir.dt.bfloat16
FP8 = mybir.dt.float8e4
AF = mybir.ActivationFunctionType
ALU = mybir.AluOpType
DR = mybir.MatmulPerfMode.DoubleRow
E4NP = ml_dtypes.float8_e4m3

B, N, C = 8, 1024, 768
GD = 384
H, D = 6, 64
DP = D + 1              # v head dim padded with ones column
SCALE = D ** -0.5
HID = 3072
EPS = 1e-6
NH = 2                  # token n-halves of 512
NHW = N // NH           # 512
MC = N // 128           # 8 token chunks
CC = C // 128           # 6 feature chunks
GC = GD // 128          # 3 feature chunks per branch
JC = HID // 128         # 24 hidden chunks
WS = 1024.0             # weight quant scale (2^10)
QS = 2.0 ** -4          # q/k/v psum -> fp8 rescale (carries 2^6)
DQ_PROJ = 2.0 ** -16    # proj psum dequant (oT 2^6 * W 2^10)
DQ_FC = 2.0 ** -10      # fc psum dequant (acts true-scale, W 2^10)
EXP_SCALE_G = SCALE * 2.0 ** -12  # global: q,k each carry 2^6
EXP_SCALE_L = SCALE * 2.0 ** -20  # local: ql,kl each carry 2^10


def f32(ap):
    return ap.bitcast(F32)


def _build(flags):
    nc = bacc.Bacc("TRN2", target_bir_lowering=False, debug=False)

    x_d = nc.dram_tensor("x", (N, C), F32, kind="ExternalInput")
    gqk8_d = nc.dram_tensor("gqk8", (GD, 2 * GD), FP8, kind="ExternalInput")
    wv8_d = nc.dram_tensor("wv8", (GD, H * DP), FP8, kind="ExternalInput")
    lqkv8_d = nc.dram_tensor("lqkv8", (GD, 3 * GD), FP8, kind="ExternalInput")
    gp8_d = nc.dram_tensor("gp8", (GD, GD), FP8, kind="ExternalInput")
    lp8_d = nc.dram_tensor("lp8", (GD, GD), FP8, kind="ExternalInput")
    fc1h_d = nc.dram_tensor("fc1h", (C, HID), FP8, kind="ExternalInput")
    fc1l_d = nc.dram_tensor("fc1l", (C, HID), FP8, kind="ExternalInput")
    fc2h_d = nc.dram_tensor("fc2h", (HID, C), FP8, kind="ExternalInput")
    fc2l_d = nc.dram_tensor("fc2l", (HID, C), FP8, kind="ExternalInput")
    opt = {}
    for nm, sz, fl in (("ln1_g", GD, "gb1g"), ("ln1_b", GD, "gb1g"),
                       ("ln1l_g", GD, "gb1l"), ("ln1l_b", GD, "gb1l"),
                       ("ln2_g", C, "gb2"), ("ln2_b", C, "gb2"),
                       ("g_proj_b", GD, "bias_gproj"), ("l_proj_b", GD, "bias_lproj"),
                       ("fc1_b", HID, "bias_fc1"), ("fc2_b", C, "bias_fc2")):
        if flags[fl]:
            opt[nm] = nc.dram_tensor(nm, (sz,), F32, kind="ExternalInput")
    out_d = nc.dram_tensor("out", (N, C), F32, kind="ExternalOutput")

    gqk8_v = gqk8_d.rearrange("(kc p) c -> p kc c", p=128)
    wv8_v = wv8_d.rearrange("(kc p) c -> p kc c", p=128)
    lqkv8_v = lqkv8_d.rearrange("(kc p) c -> p kc c", p=128)
    gp8_v = gp8_d.rearrange("(kc p) c -> p kc c", p=128)
    lp8_v = lp8_d.rearrange("(kc p) c -> p kc c", p=128)
    fc1h_v = fc1h_d.rearrange("(kc p) c -> p kc c", p=128)
    fc1l_v = fc1l_d.rearrange("(kc p) c -> p kc c", p=128)
    fc2h_v = fc2h_d.rearrange("(kc p) c -> p kc c", p=128)
    fc2l_v = fc2l_d.rearrange("(kc p) c -> p kc c", p=128)

    with tile.TileContext(nc) as tc, ExitStack() as top:
        consts = top.enter_context(tc.tile_pool(name="consts", bufs=1))
        core = top.enter_context(tc.tile_pool(name="core", bufs=1))
        wpool = top.enter_context(tc.tile_pool(name="wpool", bufs=1))

        identB = consts.tile([128, 128], BF16, tag="identB")
        make_identity(nc, identB)
        onesR = consts.tile([128, 1], F32, tag="onesR")
        nc.vector.memset(onesR, 1.0)
        onesR = onesR.bitcast(F32R)
        onesB2 = consts.tile([128, 1], BF16, tag="onesB2")
        nc.vector.memset(onesB2, 1.0)
        onesRow = consts.tile([1, 128], BF16, tag="onesRow")
        nc.vector.memset(onesRow, 1.0)
        c1row = consts.tile([1, 64], BF16, tag="c1row")
        nc.vector.memset(c1row, 1.0)
        eps_t = consts.tile([1, 1], F32, tag="eps")
        nc.vector.memset(eps_t, EPS)
        # blkS[p, kc, j]: headsum lhsT (1 if j == 2*kc + p//64)
        blkS = consts.tile([128, GC, 2 * GC], BF16, tag="blkS")
        nc.vector.memset(blkS, 0.0)
        for kc in range(GC):
            nc.vector.memset(blkS[0:64, kc, 2 * kc:2 * kc + 1], 1.0)
            nc.vector.memset(blkS[64:128, kc, 2 * kc + 1:2 * kc + 2], 1.0)
        # blkT[j, kc, p]: head->partition broadcast lhsT (j = head index 0..5)
        blkT = consts.tile([H, GC, 128], BF16, tag="blkT")
        nc.vector.memset(blkT, 0.0)
        for kc in range(GC):
            nc.vector.memset(blkT[2 * kc:2 * kc + 1, kc, 0:64], 1.0)
            nc.vector.memset(blkT[2 * kc + 1:2 * kc + 2, kc, 64:128], 1.0)

        def load_vec(dram, n_elems, tag):
            t = consts.tile([128, n_elems // 128], F32, tag=tag)
            nc.sync.dma_start(t, dram.rearrange("(c p) -> p c", p=128))
            return t

        g1g = load_vec(opt["ln1_g"], GD, "g1g") if flags["gb1g"] else None
        g1b = load_vec(opt["ln1_b"], GD, "g1b") if flags["gb1g"] else None
        l1g = load_vec(opt["ln1l_g"], GD, "l1g") if flags["gb1l"] else None
        l1b = load_vec(opt["ln1l_b"], GD, "l1b") if flags["gb1l"] else None
        g2g = load_vec(opt["ln2_g"], C, "g2g") if flags["gb2"] else None
        g2b = load_vec(opt["ln2_b"], C, "g2b") if flags["gb2"] else None
        gpb = load_vec(opt["g_proj_b"], GD, "gpb") if flags["bias_gproj"] else None
        lpb = load_vec(opt["l_proj_b"], GD, "lpb") if flags["bias_lproj"] else None
        fc1b = load_vec(opt["fc1_b"], HID, "fc1b") if flags["bias_fc1"] else None
        fc2b = load_vec(opt["fc2_b"], C, "fc2b") if flags["bias_fc2"] else None

        # resident fp8 weights (prefetched at t0, no conversion needed)
        gqk8 = wpool.tile([128, GC, 2 * GD], FP8, tag="gqk8")
        nc.sync.dma_start(gqk8, gqk8_v)
        wv8 = wpool.tile([128, GC, H * DP], FP8, tag="wv8")
        nc.sync.dma_start(wv8, wv8_v)
        lqkv8 = wpool.tile([128, GC, 3 * GD], FP8, tag="lqkv8")
        nc.sync.dma_start(lqkv8, lqkv8_v)
        gp8 = wpool.tile([128, GC, GD], FP8, tag="gp8")
        nc.sync.dma_start(gp8, gp8_v)
        lp8 = wpool.tile([128, GC, GD], FP8, tag="lp8")
        nc.sync.dma_start(lp8, lp8_v)
        fc1h = wpool.tile([128, CC, HID], FP8, tag="fc1h")
        fc1l = wpool.tile([128, CC, HID], FP8, tag="fc1l")
        fc2h = wpool.tile([128, JC, C], FP8, tag="fc2h")
        fc2l = wpool.tile([128, JC, C], FP8, tag="fc2l")
        for kc in range(CC):
            nc.sync.dma_start(fc1h[:, kc], fc1h_v[:, kc])
            nc.sync.dma_start(fc1l[:, kc], fc1l_v[:, kc])
        for kc in range(0, JC, 4):
            nc.sync.dma_start(fc2h[:, kc:kc + 4], fc2h_v[:, kc:kc + 4])
            nc.sync.dma_start(fc2l[:, kc:kc + 4], fc2l_v[:, kc:kc + 4])

        xT = core.tile([128, CC, N], F32R, tag="xT")   # residual, feature-major

        # ---------------- phase A: load x, transpose to feature-major --------
        with tc.tile_pool(name="xtok", bufs=3) as xtok_p, \
             tc.tile_pool(name="ps_tr0", bufs=3, space="PSUM") as ps_tr0:
            for m in range(MC):
                xt = xtok_p.tile([128, C], F32, tag="xt")
                nc.sync.dma_start(xt, x_d[m * 128:(m + 1) * 128, :])
                xtr = xt.bitcast(F32R)
                for cq in range(CC // 2):
                    ps = ps_tr0.tile([128, 2, 128], F32R, tag="tr")
                    for half in range(2):
                        c = 2 * cq + half
                        nc.tensor.transpose(ps[:, half], xtr[:, c * 128:(c + 1) * 128],
                                            identB)
                    dst = xT[:, 2 * cq:2 * cq + 2, m * 128:(m + 1) * 128]
                    if (m + cq) % 2 == 0:
                        nc.vector.tensor_copy(f32(dst), f32(ps))
                    else:
                        nc.scalar.copy(f32(dst), f32(ps))

        # ---------------- feature-major LayerNorm helper ----------------
        def ln_feat(lo, hi, dst, gv, bv, sq_p, st_p, bc_p, tmp_p, sq_eng):
            """dst[:, c-lo, :] = fp8(LN(xT rows [lo*128, hi*128)) over features)."""
            nch = hi - lo
            inv = 1.0 / (nch * 128)
            for nh in range(NH):
                ns = slice(nh * NHW, (nh + 1) * NHW)
                st = st_p.tile([1, 2 * NHW], F32, tag="stat")
                for i, c in enumerate(range(lo, hi)):
                    nc.tensor.matmul(st[:, 0:NHW], onesR[:, 0:1], xT[:, c, ns],
                                     start=(i == 0), stop=(i == nch - 1))
                for i, c in enumerate(range(lo, hi)):
                    sq = sq_p.tile([128, NHW], BF16, tag="sq")
                    if sq_eng == "act":
                        nc.scalar.activation(sq, f32(xT[:, c, ns]), AF.Square)
                    else:
                        nc.gpsimd.tensor_tensor(sq, f32(xT[:, c, ns]),
                                                f32(xT[:, c, ns]), ALU.mult)
                    nc.tensor.matmul(st[:, NHW:2 * NHW], onesB2[:, 0:1], sq,
                                     start=(i == 0), stop=(i == nch - 1))
                mean = sq_p.tile([1, NHW], F32, tag="mean")
                nc.vector.tensor_scalar_mul(mean, st[:, 0:NHW], inv)
                e2 = sq_p.tile([1, NHW], F32, tag="e2")
                nc.vector.tensor_scalar_mul(e2, st[:, NHW:2 * NHW], inv)
                var = sq_p.tile([1, NHW], F32, tag="var")
                nc.vector.tensor_tensor(var, mean, mean, ALU.mult)
                nc.vector.tensor_tensor(var, e2, var, ALU.subtract)
                sr = sq_p.tile([1, NHW], F32, tag="sr")
                nc.scalar.activation(sr, var, AF.Sqrt, bias=eps_t[0:1, :], scale=1.0)
                r_bf = sq_p.tile([1, NHW], BF16, tag="r_bf")
                with nc.allow_low_precision(reason="bf16 rstd for bcast matmul"):
                    nc.vector.reciprocal(r_bf, sr)
                mr_bf = sq_p.tile([1, NHW], BF16, tag="mr_bf")
                nc.vector.tensor_tensor(mr_bf, mean, r_bf, ALU.mult)
                rB = bc_p.tile([128, NHW], F32, tag="rB")
                nc.tensor.matmul(rB, onesRow, r_bf, start=True, stop=True)
                mrB = bc_p.tile([128, NHW], F32, tag="mrB")
                nc.tensor.matmul(mrB, onesRow, mr_bf, start=True, stop=True)
                for c in range(lo, hi):
                    eng = nc.vector if (c % 2 == 0) else nc.gpsimd
                    t = tmp_p.tile([128, NHW], F32, tag="xnorm")
                    eng.tensor_tensor(t, f32(xT[:, c, ns]), rB, ALU.mult)
                    dslice = dst[:, c - lo, ns]
                    if gv is not None:
                        t2 = tmp_p.tile([128, NHW], F32, tag="xnorm2")
                        eng.tensor_tensor(t2, t, mrB, ALU.subtract)
                        eng.tensor_scalar(dslice, t2, gv[:, c - lo:c - lo + 1],
                                          bv[:, c - lo:c - lo + 1], ALU.mult, ALU.add)
                    else:
                        eng.tensor_tensor(dslice, t, mrB, ALU.subtract)

        # ---------------- phase B: LN1 (both branches) ----------------
        xgln = core.tile([128, GC, N], FP8, tag="xgln")
        xlln = core.tile([128, GC, N], FP8, tag="xlln")
        with tc.tile_pool(name="sq1", bufs=4) as sq_p, \
             tc.tile_pool(name="tmp1", bufs=4) as tmp_p, \
             tc.tile_pool(name="st1", bufs=1, space="PSUM") as st_p, \
             tc.tile_pool(name="bc1", bufs=2, space="PSUM") as bc_p:
            ln_feat(0, GC, xgln, g1g, g1b, sq_p, st_p, bc_p, tmp_p, "act")
            ln_feat(GC, CC, xlln, l1g, l1b, sq_p, st_p, bc_p, tmp_p, "act")

        # DR contraction helper over GC=3 chunks: pair (0,1) + single 2
        def mm3(ps, w, rhs_t, cols, ns):
            nc.tensor.matmul(ps, w[:, 0:2, cols], rhs_t[:, 0:2, ns],
                             start=True, stop=False, perf_mode=DR)
            nc.tensor.matmul(ps, w[:, 2, cols], rhs_t[:, 2, ns],
                             start=False, stop=True)

        # ---------------- phases C+D: attention (global + dripped local) -----
        qT = core.tile([128, GC, N], FP8, tag="qT")      # x2^6
        kT = core.tile([128, GC, N], FP8, tag="kT")      # x2^6
        vpad = core.tile([128, MC, H * DP], FP8, tag="vpad")  # x2^6, ones col
        oT = core.tile([128, GC, N], FP8, tag="oT")      # x2^6
        qlT = core.tile([128, GC, N], BF16, tag="qlT")   # x2^10
        klT = core.tile([128, GC, N], BF16, tag="klT")   # x2^10
        vlT = core.tile([128, GC, N], FP8, tag="vlT")    # x2^6
        oTl = core.tile([128, GC, N], FP8, tag="oTl")    # x2^6

        with tc.tile_pool(name="esb", bufs=3) as e_p, \
             tc.tile_pool(name="small", bufs=3) as sm_p, \
             tc.tile_pool(name="pqk", bufs=2, space="PSUM") as pq_p, \
             tc.tile_pool(name="psc", bufs=2, space="PSUM") as ps_p, \
             tc.tile_pool(name="po", bufs=1, space="PSUM") as po_p, \
             tc.tile_pool(name="pb", bufs=1, space="PSUM") as pb_p:

            # global q,k (x2^6 after QS rescale of x2^10 psum)
            for mo in range(2 * GC):
                dst = qT if mo < GC else kT
                dc = mo % GC
                for nh in range(NH):
                    ns = slice(nh * NHW, (nh + 1) * NHW)
                    ps = pq_p.tile([128, NHW], F32, tag="pq")
                    mm3(ps, gqk8, xgln, slice(mo * 128, (mo + 1) * 128), ns)
                    nc.vector.tensor_scalar_mul(dst[:, dc, ns], ps, QS)
            # global v (token-major, x2^6), ones col after
            for m in range(MC):
                ps = pq_p.tile([128, NHW], F32, tag="pq")
                psv = ps[:, 0:H * DP]
                nc.tensor.matmul(psv, xgln[:, 0:2, m * 128:(m + 1) * 128],
                                 wv8[:, 0:2, :], start=True, stop=False, perf_mode=DR)
                nc.tensor.matmul(psv, xgln[:, 2, m * 128:(m + 1) * 128],
                                 wv8[:, 2, :], start=False, stop=True)
                nc.gpsimd.tensor_scalar_mul(vpad[:, m, :], psv, QS)
            nc.vector.memset(
                vpad.rearrange("p m (h e) -> p m h e", e=DP)[:, :, :, D:D + 1], 1.0)

            # local qkv drip units (run inside the scores loop on PE gaps)
            lq_units = []
            for pi in range(3):
                for oc in range(GC):
                    for nh in range(NH):
                        lq_units.append((pi, oc, nh))

            def emit_lq(n):
                for _ in range(n):
                    if not lq_units:
                        return
                    pi, oc, nh = lq_units.pop(0)
                    ns = slice(nh * NHW, (nh + 1) * NHW)
                    ps = pq_p.tile([128, NHW], F32, tag="pq", name="lq")
                    mm3(ps, lqkv8, xlln,
                        slice(pi * GD + oc * 128, pi * GD + (oc + 1) * 128), ns)
                    if pi == 0:
                        nc.vector.tensor_copy(qlT[:, oc, ns], ps)
                    elif pi == 1:
                        nc.vector.tensor_copy(klT[:, oc, ns], ps)
                    else:
                        nc.gpsimd.tensor_scalar_mul(vlT[:, oc, ns], ps, QS)

            # scores -> exp -> DoubleRow AV, per (head, n-half)
            for h in range(H):
                hc, hp = h // 2, (h % 2) * 64
                for nh in range(NH):
                    ns = slice(nh * NHW, (nh + 1) * NHW)
                    po = po_p.tile([DP, NHW], F32, tag="po")
                    for mp in range(MC // 2):
                        ps = ps_p.tile([128, 2, NHW], F32, tag="ps")
                        for half in range(2):
                            m = 2 * mp + half
                            nc.tensor.matmul(ps[:, half],
                                             kT[hp:hp + 64, hc, m * 128:(m + 1) * 128],
                                             qT[hp:hp + 64, hc, ns],
                                             start=True, stop=True)
                        e_sb = e_p.tile([128, 2, NHW], FP8, tag="e")
                        nc.scalar.activation(
                            e_sb.rearrange("p a b -> p (a b)"),
                            ps.rearrange("p a b -> p (a b)"), AF.Exp,
                            scale=EXP_SCALE_G)
                        nc.tensor.matmul(po, vpad[:, 2 * mp:2 * mp + 2,
                                                  h * DP:(h + 1) * DP],
                                         e_sb, start=(mp == 0), stop=(mp == MC // 2 - 1),
                                         perf_mode=DR)
                    rcp = sm_p.tile([1, NHW], BF16, tag="rcp")
                    with nc.allow_low_precision(reason="bf16 recip for bcast"):
                        nc.vector.reciprocal(rcp, po[D:D + 1, :])
                    pb = pb_p.tile([64, NHW], F32, tag="pb")
                    nc.tensor.matmul(pb, c1row, rcp, start=True, stop=True)
                    pb_sb = sm_p.tile([64, NHW], BF16, tag="pbsb")
                    nc.gpsimd.tensor_copy(out=pb_sb, in_=pb)
                    nc.vector.tensor_tensor(oT[hp:hp + 64, hc, ns], po[0:D, :],
                                            pb_sb, ALU.mult)
                emit_lq(2)
            emit_lq(len(lq_units))

        # ---------------- phase E: projections + local attention -------------
        with tc.tile_pool(name="lwork", bufs=1) as lw_p, \
             tc.tile_pool(name="ltmp", bufs=3) as lt_p, \
             tc.tile_pool(name="pesc", bufs=1, space="PSUM") as pe_p, \
             tc.tile_pool(name="pab", bufs=2, space="PSUM") as pa_p, \
             tc.tile_pool(name="ppr", bufs=2, space="PSUM") as pp_p:

            # local banded products (shifts are free-axis slices)
            prod_m = lw_p.tile([128, GC, N], BF16, tag="prodm")
            prod_0 = lw_p.tile([128, GC, N], BF16, tag="prod0")
            prod_p = lw_p.tile([128, GC, N], BF16, tag="prodp")
            nc.vector.memset(prod_m[:, :, 0:1], 0.0)
            nc.vector.memset(prod_p[:, :, N - 1:N], 0.0)
            nc.vector.tensor_tensor(prod_m[:, :, 1:N], qlT[:, :, 1:N],
                                    klT[:, :, 0:N - 1], ALU.mult)
            nc.vector.tensor_tensor(prod_0, qlT, klT, ALU.mult)
            nc.vector.tensor_tensor(prod_p[:, :, 0:N - 1], qlT[:, :, 0:N - 1],
                                    klT[:, :, 1:N], ALU.mult)

            # global proj + residual into xT rows [0, GD)
            for mo in range(GC):
                for nh in range(NH):
                    ns = slice(nh * NHW, (nh + 1) * NHW)
                    ps = pp_p.tile([128, NHW], F32, tag="ppr")
                    mm3(ps, gp8, oT, slice(mo * 128, (mo + 1) * 128), ns)
                    if gpb is not None:
                        nc.scalar.activation(ps, ps, AF.Identity,
                                             bias=gpb[:, mo:mo + 1], scale=DQ_PROJ)
                        nc.vector.tensor_tensor(xT[:, mo, ns], f32(xT[:, mo, ns]),
                                                ps, ALU.add)
                    else:
                        nc.vector.scalar_tensor_tensor(
                            xT[:, mo, ns], ps, DQ_PROJ, f32(xT[:, mo, ns]),
                            ALU.mult, ALU.add)

            # local head-sums -> per-shift escore psum [6, 512] (x2^20), softmax
            aw = lw_p.tile([H, 3, N], BF16, tag="aw")
            for nh in range(NH):
                ns = slice(nh * NHW, (nh + 1) * NHW)
                ees = []
                for si, prod in enumerate((prod_m, prod_0, prod_p)):
                    esc = pe_p.tile([H, NHW], F32, tag=f"esc{si}", name=f"esc{si}")
                    for kc in range(GC):
                        nc.tensor.matmul(esc, blkS[:, kc, :], prod[:, kc, ns],
                                         start=(kc == 0), stop=(kc == GC - 1))
                    # mask band edges (exp -> 0)
                    if si == 0 and nh == 0:
                        nc.vector.memset(esc[:, 0:1], -1e30)
                    if si == 2 and nh == NH - 1:
                        nc.vector.memset(esc[:, NHW - 1:NHW], -1e30)
                    ee = lt_p.tile([H, NHW], BF16, tag="ee")
                    nc.scalar.activation(ee, esc, AF.Exp, scale=EXP_SCALE_L)
                    ees.append(ee)
                dsum = lt_p.tile([H, NHW], BF16, tag="dsum")
                nc.vector.tensor_tensor(dsum, ees[0], ees[1], ALU.add)
                nc.vector.tensor_tensor(dsum, dsum, ees[2], ALU.add)
                rr = lt_p.tile([H, NHW], BF16, tag="rr")
                with nc.allow_low_precision(reason="bf16 softmax recip"):
                    nc.vector.reciprocal(rr, dsum)
                for si in range(3):
                    nc.vector.tensor_tensor(aw[:, si, ns], ees[si], rr, ALU.mult)

            # o accum in bf16; aB broadcast via PE; final (s=0) add writes oTl fp8
            o_acc = lw_p.tile([128, GC, N], BF16, tag="oacc")
            nc.vector.memset(o_acc[:, :, 0:1], 0.0)
            nc.vector.memset(o_acc[:, :, N - 1:N], 0.0)

            def avpass(si, first, last):
                # o += vlT(shifted) * broadcast(aw[si]); s=0 pass writes oTl
                for kc in range(GC):
                    for nh in range(NH):
                        ns = slice(nh * NHW, (nh + 1) * NHW)
                        aB = pa_p.tile([128, NHW], F32, tag="aB")
                        nc.tensor.matmul(aB, blkT[:, kc, :], aw[:, si, ns],
                                         start=True, stop=True)
                        lo_n = nh * NHW
                        hi_n = (nh + 1) * NHW
                        if si == 0:
                            vs, ve = max(lo_n, 1) - 1, hi_n - 1
                            os_, oe = max(lo_n, 1), hi_n
                        elif si == 2:
                            vs, ve = lo_n + 1, min(hi_n, N - 1) + 1
                            os_, oe = lo_n, min(hi_n, N - 1)
                        else:
                            vs, ve, os_, oe = lo_n, hi_n, lo_n, hi_n
                        a_sl = aB[:, os_ - lo_n:oe - lo_n]
                        v_sl = vlT[:, kc, vs:ve]
                        eng = nc.gpsimd if si == 2 else nc.vector
                        if first:
                            eng.tensor_tensor(o_acc[:, kc, os_:oe], v_sl, a_sl,
                                              ALU.mult)
                        elif not last:
                            t = lt_p.tile([128, NHW], BF16, tag="avt")
                            eng.tensor_tensor(t[:, 0:oe - os_], v_sl, a_sl, ALU.mult)
                            eng.tensor_tensor(o_acc[:, kc, os_:oe],
                                              o_acc[:, kc, os_:oe],
                                              t[:, 0:oe - os_], ALU.add)
                        else:
                            t = lt_p.tile([128, NHW], BF16, tag="avt")
                            eng.tensor_tensor(t[:, 0:oe - os_], v_sl, a_sl, ALU.mult)
                            eng.tensor_tensor(oTl[:, kc, os_:oe],
                                              o_acc[:, kc, os_:oe],
                                              t[:, 0:oe - os_], ALU.add)

            avpass(0, True, False)   # s=-1 writes o_acc
            avpass(2, False, False)  # s=+1 adds
            avpass(1, False, True)   # s=0 adds, writes oTl (full range)

            # local proj + residual into xT rows [GD, C)
            for mo in range(GC):
                for nh in range(NH):
                    ns = slice(nh * NHW, (nh + 1) * NHW)
                    ps = pp_p.tile([128, NHW], F32, tag="ppr")
                    mm3(ps, lp8, oTl, slice(mo * 128, (mo + 1) * 128), ns)
                    if lpb is not None:
                        nc.scalar.activation(ps, ps, AF.Identity,
                                             bias=lpb[:, mo:mo + 1], scale=DQ_PROJ)
                        nc.vector.tensor_tensor(xT[:, GC + mo, ns],
                                                f32(xT[:, GC + mo, ns]), ps, ALU.add)
                    else:
                        nc.vector.scalar_tensor_tensor(
                            xT[:, GC + mo, ns], ps, DQ_PROJ,
                            f32(xT[:, GC + mo, ns]), ALU.mult, ALU.add)

        # ---------------- phase F: LN2 ----------------
        hT = core.tile([128, CC, N], FP8, tag="hT")
        with tc.tile_pool(name="sq2", bufs=4) as sq_p, \
             tc.tile_pool(name="tmp2", bufs=4) as tmp_p, \
             tc.tile_pool(name="st2", bufs=1, space="PSUM") as st_p, \
             tc.tile_pool(name="bc2", bufs=2, space="PSUM") as bc_p:
            ln_feat(0, CC, hT, g2g, g2b, sq_p, st_p, bc_p, tmp_p, "pool")

        # ---------------- phase G: MLP + transpose out ----------------
        with tc.tile_pool(name="gl", bufs=1) as gl_pool, \
             tc.tile_pool(name="otok", bufs=2) as otok_p, \
             tc.tile_pool(name="outT", bufs=1) as outT_p, \
             tc.tile_pool(name="pm", bufs=2, space="PSUM") as pm_p, \
             tc.tile_pool(name="pz", bufs=2, space="PSUM") as pz_p, \
             tc.tile_pool(name="ps_tr3", bufs=2, space="PSUM") as ps_tr3:
            gls = [gl_pool.tile([128, 2, NHW], FP8, tag=f"gl{jp}", name=f"gl{jp}")
                   for jp in range(JC // 2)]
            for nh in range(NH):
                ns = slice(nh * NHW, (nh + 1) * NHW)
                outT = outT_p.tile([128, CC, NHW], F32, tag="outT")
                # fc1 (hi+lo DR) -> gelu(pair) -> gl fp8
                for jp in range(JC // 2):
                    pm = pm_p.tile([128, 2, NHW], F32, tag="pm")
                    for half in range(2):
                        j = 2 * jp + half
                        js = slice(j * 128, (j + 1) * 128)
                        for t in range(CC // 2):
                            nc.tensor.matmul(pm[:, half], fc1h[:, 2 * t:2 * t + 2, js],
                                             hT[:, 2 * t:2 * t + 2, ns],
                                             start=(t == 0), stop=False, perf_mode=DR)
                        for t in range(CC // 2):
                            nc.tensor.matmul(pm[:, half], fc1l[:, 2 * t:2 * t + 2, js],
                                             hT[:, 2 * t:2 * t + 2, ns],
                                             start=False, stop=(t == CC // 2 - 1),
                                             perf_mode=DR)
                    gl = gls[jp]
                    if fc1b is not None:
                        # bias is per hidden unit = per psum partition, halves differ
                        for half in range(2):
                            j = 2 * jp + half
                            nc.scalar.activation(gl[:, half], pm[:, half], AF.Gelu,
                                                 bias=fc1b[:, j:j + 1], scale=DQ_FC)
                    else:
                        nc.scalar.activation(gl.rearrange("p a b -> p (a b)"),
                                             pm.rearrange("p a b -> p (a b)"),
                                             AF.Gelu, scale=DQ_FC)
                # fc2 (hi+lo DR) per output chunk, then residual + transpose
                for mo in range(CC):
                    cs = slice(mo * 128, (mo + 1) * 128)
                    zp = pz_p.tile([128, NHW], F32, tag="pz")
                    for jp in range(JC // 2):
                        nc.tensor.matmul(zp, fc2h[:, 2 * jp:2 * jp + 2, cs], gls[jp],
                                         start=(jp == 0), stop=False, perf_mode=DR)
                    for jp in range(JC // 2):
                        nc.tensor.matmul(zp, fc2l[:, 2 * jp:2 * jp + 2, cs], gls[jp],
                                         start=False, stop=(jp == JC // 2 - 1),
                                         perf_mode=DR)
                    if fc2b is not None:
                        nc.scalar.activation(zp, zp, AF.Identity,
                                             bias=fc2b[:, mo:mo + 1], scale=DQ_FC)
                        nc.vector.tensor_tensor(outT[:, mo], f32(xT[:, mo, ns]),
                                                zp, ALU.add)
                    else:
                        nc.vector.scalar_tensor_tensor(
                            outT[:, mo], zp, DQ_FC, f32(xT[:, mo, ns]),
                            ALU.mult, ALU.add)
                # transpose out + store (4 m-chunks per half)
                outTr = outT.bitcast(F32R)
                for mq in range(NHW // 128):
                    ot = otok_p.tile([128, C], F32, tag="ot")
                    for cq in range(CC // 2):
                        ps = ps_tr3.tile([128, 2, 128], F32R, tag="tr3")
                        for half in range(2):
                            c = 2 * cq + half
                            nc.tensor.transpose(ps[:, half],
                                                outTr[:, c, mq * 128:(mq + 1) * 128],
                                                identB)
                        dst = ot[:, 2 * cq * 128:(2 * cq + 2) * 128]
                        dst = dst.rearrange("p (a b) -> p a b", a=2)
                        if (mq + cq) % 2 == 0:
                            nc.gpsimd.tensor_copy(out=dst, in_=f32(ps))
                        else:
                            nc.scalar.copy(dst, f32(ps))
                    tok0 = nh * NHW + mq * 128
                    nc.sync.dma_start(out_d[tok0:tok0 + 128, :], ot)

    nc.compile()
    return nc


_NC_CACHE = {}


def _q8(w, s=WS):
    return np.clip(w.astype(np.float64) * s, -240.0, 240.0).astype(E4NP)


def _q8_split(w, s=WS):
    ws = np.clip(w.astype(np.float64) * s, -240.0, 240.0)
    hi = ws.astype(E4NP)
    lo = np.clip(ws - hi.astype(np.float64), -240.0, 240.0).astype(E4NP)
    return hi, lo


def kernel(**inputs):
    inp = {k: np.ascontiguousarray(np.asarray(v), dtype=np.float32)
           for k, v in inputs.items()}
    flags = {
        "gb1g": not (np.all(inp["ln1_g"] == 1.0) and np.all(inp["ln1_b"] == 0.0)),
        "gb1l": not (np.all(inp["ln1l_g"] == 1.0) and np.all(inp["ln1l_b"] == 0.0)),
        "gb2": not (np.all(inp["ln2_g"] == 1.0) and np.all(inp["ln2_b"] == 0.0)),
        "bias_gproj": bool(np.any(inp["g_proj_b"] != 0.0)),
        "bias_lproj": bool(np.any(inp["l_proj_b"] != 0.0)),
        "bias_fc1": bool(np.any(inp["fc1_b"] != 0.0)),
        "bias_fc2": bool(np.any(inp["fc2_b"] != 0.0)),
    }
    key = tuple(sorted(flags.items()))
    nc = _NC_CACHE.get(key)
    if nc is None:
        nc = _build(flags)
        _NC_CACHE[key] = nc

    g_qkv = inp["g_qkv_w"]
    wv = np.zeros((GD, H * DP), np.float32)
    wv.reshape(GD, H, DP)[:, :, :D] = g_qkv[:, 2 * GD:].reshape(GD, H, D)
    fc1h, fc1l = _q8_split(inp["fc1_w"])
    fc2h, fc2l = _q8_split(inp["fc2_w"])
    weights = {
        "gqk8": _q8(g_qkv[:, :2 * GD]),
        "wv8": _q8(wv),
        "lqkv8": _q8(inp["l_qkv_w"]),
        "gp8": _q8(inp["g_proj_w"]),
        "lp8": _q8(inp["l_proj_w"]),
        "fc1h": fc1h, "fc1l": fc1l, "fc2h": fc2h, "fc2l": fc2l,
    }
    for nm, fl in (("ln1_g", "gb1g"), ("ln1_b", "gb1g"), ("ln1l_g", "gb1l"),
                   ("ln1l_b", "gb1l"), ("ln2_g", "gb2"), ("ln2_b", "gb2"),
                   ("g_proj_b", "bias_gproj"), ("l_proj_b", "bias_lproj"),
                   ("fc1_b", "bias_fc1"), ("fc2_b", "bias_fc2")):
        if flags[fl]:
            weights[nm] = inp[nm]

    x = inp["x"]
    in_maps = [dict(weights, x=np.ascontiguousarray(x[b])) for b in range(B)]
    res = run_bass_kernel_spmd(nc, in_maps, core_ids=list(range(B)))
    return np.stack([res.results[b]["out"] for b in range(B)]).astype(np.float32)


# revision 19
# speedup vs baseline: 1.4708x; 1.0869x over previous
"""Trainium2 Bass kernel for nn_Block_local (dual global/banded-local attention block).

Sharding: data-parallel, one batch element per NeuronCore (B=8, 8 cores).
Feature-major activations ([C,N]); fp8e4 DoubleRow matmuls for all
weight-contractions (weights quantized host-side, fc1/fc2 split hi+lo fp8),
bf16 scores, fp8 softmax/activation intermediates, feature-major banded local
attention (shifts are free-axis slices; no shift DMAs, no local transposes).
"""
import os
import numpy as np
import ml_dtypes

import concourse.bass as bass
import concourse.bacc as bacc
import concourse.mybir as mybir
import concourse.tile as tile
from concourse.bass_utils import run_bass_kernel_spmd
from concourse.masks import make_identity
from contextlib import ExitStack

F32 = mybir.dt.float32
F32R = mybir.dt.float32r
BF16 = mybir.dt.bfloat16
FP8 = mybir.dt.float8e4
AF = mybir.ActivationFunctionType
ALU = mybir.AluOpType
DR = mybir.MatmulPerfMode.DoubleRow
E4NP = ml_dtypes.float8_e4m3

B, N, C = 8, 1024, 768
GD = 384
H, D = 6, 64
DP = D + 1              # v head dim padded with ones column
SCALE = D ** -0.5
HID = 3072
EPS = 1e-6
NH = 2                  # token n-halves of 512
NHW = N // NH           # 512
MC = N // 128           # 8 token chunks
CC = C // 128           # 6 feature chunks
GC = GD // 128          # 3 feature chunks per branch
JC = HID // 128         # 24 hidden chunks
WS = 1024.0             # weight quant scale (2^10)
QS = 2.0 ** -4          # q/k/v psum -> fp8 rescale (carries 2^6)
DQ_PROJ = 2.0 ** -16    # proj psum dequant (oT 2^6 * W 2^10)
DQ_FC = 2.0 ** -10      # fc psum dequant (acts true-scale, W 2^10)
EXP_SCALE_G = SCALE * 2.0 ** -12  # global: q,k each carry 2^6
EXP_SCALE_L = SCALE * 2.0 ** -20  # local: ql,kl each carry 2^10


def f32(ap):
    return ap.bitcast(F32)


def _build(flags):
    nc = bacc.Bacc("TRN2", target_bir_lowering=False, debug=False)

    x_d = nc.dram_tensor("x", (N, C), F32, kind="ExternalInput")
    gqk8_d = nc.dram_tensor("gqk8", (GD, 2 * GD), FP8, kind="ExternalInput")
    wv8_d = nc.dram_tensor("wv8", (GD, H * DP), FP8, kind="ExternalInput")
    lqkv8_d = nc.dram_tensor("lqkv8", (GD, 3 * GD), FP8, kind="ExternalInput")
    gp8_d = nc.dram_tensor("gp8", (GD, GD), FP8, kind="ExternalInput")
    lp8_d = nc.dram_tensor("lp8", (GD, GD), FP8, kind="ExternalInput")
    fc1h_d = nc.dram_tensor("fc1h", (C, HID), FP8, kind="ExternalInput")
    fc1l_d = nc.dram_tensor("fc1l", (C, HID), FP8, kind="ExternalInput")
    fc2h_d = nc.dram_tensor("fc2h", (HID, C), FP8, kind="ExternalInput")
    fc2l_d = nc.dram_tensor("fc2l", (HID, C), FP8, kind="ExternalInput")
    opt = {}
    for nm, sz, fl in (("ln1_g", GD, "gb1g"), ("ln1_b", GD, "gb1g"),
                       ("ln1l_g", GD, "gb1l"), ("ln1l_b", GD, "gb1l"),
                       ("ln2_g", C, "gb2"), ("ln2_b", C, "gb2"),
                       ("g_proj_b", GD, "bias_gproj"), ("l_proj_b", GD, "bias_lproj"),
                       ("fc1_b", HID, "bias_fc1"), ("fc2_b", C, "bias_fc2")):
        if flags[fl]:
            opt[nm] = nc.dram_tensor(nm, (sz,), F32, kind="ExternalInput")
    out_d = nc.dram_tensor("out", (N, C), F32, kind="ExternalOutput")

    gqk8_v = gqk8_d.rearrange("(kc p) c -> p kc c", p=128)
    wv8_v = wv8_d.rearrange("(kc p) c -> p kc c", p=128)
    lqkv8_v = lqkv8_d.rearrange("(kc p) c -> p kc c", p=128)
    gp8_v = gp8_d.rearrange("(kc p) c -> p kc c", p=128)
    lp8_v = lp8_d.rearrange("(kc p) c -> p kc c", p=128)
    fc1h_v = fc1h_d.rearrange("(kc p) c -> p kc c", p=128)
    fc1l_v = fc1l_d.rearrange("(kc p) c -> p kc c", p=128)
    fc2h_v = fc2h_d.rearrange("(kc p) c -> p kc c", p=128)
    fc2l_v = fc2l_d.rearrange("(kc p) c -> p kc c", p=128)

    with tile.TileContext(nc) as tc, ExitStack() as top:
        consts = top.enter_context(tc.tile_pool(name="consts", bufs=1))
        core = top.enter_context(tc.tile_pool(name="core", bufs=1))
        wpool = top.enter_context(tc.tile_pool(name="wpool", bufs=1))

        identF = consts.tile([128, 128], F32, tag="identF")
        make_identity(nc, identF)
        identR = identF.bitcast(F32R)
        onesR = consts.tile([128, 1], F32, tag="onesR")
        nc.vector.memset(onesR, 1.0)
        onesR = onesR.bitcast(F32R)
        onesB2 = consts.tile([128, 1], BF16, tag="onesB2")
        nc.vector.memset(onesB2, 1.0)
        onesRow = consts.tile([1, 128], BF16, tag="onesRow")
        nc.vector.memset(onesRow, 1.0)
        c1row = consts.tile([1, 64], BF16, tag="c1row")
        nc.vector.memset(c1row, 1.0)
        eps_t = consts.tile([1, 1], F32, tag="eps")
        nc.vector.memset(eps_t, EPS)
        # blkS[p, kc, j]: headsum lhsT (1 if j == 2*kc + p//64)
        blkS = consts.tile([128, GC, 2 * GC], BF16, tag="blkS")
        nc.vector.memset(blkS, 0.0)
        for kc in range(GC):
            nc.vector.memset(blkS[0:64, kc, 2 * kc:2 * kc + 1], 1.0)
            nc.vector.memset(blkS[64:128, kc, 2 * kc + 1:2 * kc + 2], 1.0)
        # blkT[j, kc, p]: head->partition broadcast lhsT (j = head index 0..5)
        blkT = consts.tile([H, GC, 128], BF16, tag="blkT")
        nc.vector.memset(blkT, 0.0)
        for kc in range(GC):
            nc.vector.memset(blkT[2 * kc:2 * kc + 1, kc, 0:64], 1.0)
            nc.vector.memset(blkT[2 * kc + 1:2 * kc + 2, kc, 64:128], 1.0)
        # blkT96[32*si + j, si, kc, p]: per-shift head->partition broadcast
        blkT96 = consts.tile([96, 3, GC, 128], BF16, tag="blkT96")
        nc.vector.memset(blkT96, 0.0)
        for si in range(3):
            for kc in range(GC):
                r0 = 32 * si + 2 * kc
                nc.vector.memset(blkT96[r0:r0 + 1, si, kc, 0:64], 1.0)
                nc.vector.memset(blkT96[r0 + 1:r0 + 2, si, kc, 64:128], 1.0)
        # sumInd[32*si + j, j] = 1: sums the 3 shift groups per head
        sumInd = consts.tile([96, H], BF16, tag="sumInd")
        nc.vector.memset(sumInd, 0.0)
        for si in range(3):
            for j in range(H):
                nc.vector.memset(sumInd[32 * si + j:32 * si + j + 1, j:j + 1], 1.0)

        def load_vec(dram, n_elems, tag):
            t = consts.tile([128, n_elems // 128], F32, tag=tag)
            nc.sync.dma_start(t, dram.rearrange("(c p) -> p c", p=128))
            return t

        g1g = load_vec(opt["ln1_g"], GD, "g1g") if flags["gb1g"] else None
        g1b = load_vec(opt["ln1_b"], GD, "g1b") if flags["gb1g"] else None
        l1g = load_vec(opt["ln1l_g"], GD, "l1g") if flags["gb1l"] else None
        l1b = load_vec(opt["ln1l_b"], GD, "l1b") if flags["gb1l"] else None
        g2g = load_vec(opt["ln2_g"], C, "g2g") if flags["gb2"] else None
        g2b = load_vec(opt["ln2_b"], C, "g2b") if flags["gb2"] else None
        gpb = load_vec(opt["g_proj_b"], GD, "gpb") if flags["bias_gproj"] else None
        lpb = load_vec(opt["l_proj_b"], GD, "lpb") if flags["bias_lproj"] else None
        fc1b = load_vec(opt["fc1_b"], HID, "fc1b") if flags["bias_fc1"] else None
        fc2b = load_vec(opt["fc2_b"], C, "fc2b") if flags["bias_fc2"] else None

        # resident fp8 weights (prefetched at t0, no conversion needed)
        gqk8 = wpool.tile([128, GC, 2 * GD], FP8, tag="gqk8")
        nc.gpsimd.dma_start(gqk8, gqk8_v)
        wv8 = wpool.tile([128, GC, H * DP], FP8, tag="wv8")
        nc.gpsimd.dma_start(wv8, wv8_v)
        lqkv8 = wpool.tile([128, GC, 3 * GD], FP8, tag="lqkv8")
        nc.gpsimd.dma_start(lqkv8, lqkv8_v)
        gp8 = wpool.tile([128, GC, GD], FP8, tag="gp8")
        nc.gpsimd.dma_start(gp8, gp8_v)
        lp8 = wpool.tile([128, GC, GD], FP8, tag="lp8")
        nc.gpsimd.dma_start(lp8, lp8_v)
        fc1h = wpool.tile([128, CC, HID], FP8, tag="fc1h")
        fc1l = wpool.tile([128, CC, HID], FP8, tag="fc1l")
        fc2h = wpool.tile([128, JC, C], FP8, tag="fc2h")
        fc2l = wpool.tile([128, JC, C], FP8, tag="fc2l")
        for kc in range(CC):
            nc.gpsimd.dma_start(fc1h[:, kc], fc1h_v[:, kc])
            nc.gpsimd.dma_start(fc1l[:, kc], fc1l_v[:, kc])
        for kc in range(0, JC, 4):
            nc.gpsimd.dma_start(fc2h[:, kc:kc + 4], fc2h_v[:, kc:kc + 4])
            nc.gpsimd.dma_start(fc2l[:, kc:kc + 4], fc2l_v[:, kc:kc + 4])

        xT = core.tile([128, CC, N], F32R, tag="xT")   # residual, feature-major

        # ---------------- phase A: load x, transpose to feature-major --------
        with tc.tile_pool(name="xtok", bufs=3) as xtok_p, \
             tc.tile_pool(name="ps_tr0", bufs=3, space="PSUM") as ps_tr0:
            for m in range(MC):
                xt = xtok_p.tile([128, C], F32, tag="xt")
                nc.sync.dma_start(xt, x_d[m * 128:(m + 1) * 128, :])
                xtr = xt.bitcast(F32R)
                for cq in range(CC // 2):
                    ps = ps_tr0.tile([128, 2, 128], F32R, tag="tr")
                    for half in range(2):
                        c = 2 * cq + half
                        nc.tensor.transpose(ps[:, half], xtr[:, c * 128:(c + 1) * 128],
                                            identR)
                    dst = xT[:, 2 * cq:2 * cq + 2, m * 128:(m + 1) * 128]
                    if (m + cq) % 2 == 0:
                        nc.vector.tensor_copy(f32(dst), f32(ps))
                    else:
                        nc.scalar.copy(f32(dst), f32(ps))

        # ---------------- feature-major LayerNorm helper ----------------
        def ln_feat(lo, hi, dst, gv, bv, sq_p, st_p, bc_p, tmp_p, sq_eng):
            """dst[:, c-lo, :] = fp8(LN(xT rows [lo*128, hi*128)) over features)."""
            nch = hi - lo
            inv = 1.0 / (nch * 128)
            for nh in range(NH):
                ns = slice(nh * NHW, (nh + 1) * NHW)
                st = st_p.tile([1, 2 * NHW], F32, tag="stat")
                for i, c in enumerate(range(lo, hi)):
                    nc.tensor.matmul(st[:, 0:NHW], onesR[:, 0:1], xT[:, c, ns],
                                     start=(i == 0), stop=(i == nch - 1))
                for i, c in enumerate(range(lo, hi)):
                    sq = sq_p.tile([128, NHW], BF16, tag="sq")
                    if sq_eng == "act":
                        nc.scalar.activation(sq, f32(xT[:, c, ns]), AF.Square)
                    else:
                        nc.gpsimd.tensor_tensor(sq, f32(xT[:, c, ns]),
                                                f32(xT[:, c, ns]), ALU.mult)
                    nc.tensor.matmul(st[:, NHW:2 * NHW], onesB2[:, 0:1], sq,
                                     start=(i == 0), stop=(i == nch - 1))
                mean = sq_p.tile([1, NHW], F32, tag="mean")
                nc.vector.tensor_scalar_mul(mean, st[:, 0:NHW], inv)
                e2 = sq_p.tile([1, NHW], F32, tag="e2")
                nc.vector.tensor_scalar_mul(e2, st[:, NHW:2 * NHW], inv)
                var = sq_p.tile([1, NHW], F32, tag="var")
                nc.vector.tensor_tensor(var, mean, mean, ALU.mult)
                nc.vector.tensor_tensor(var, e2, var, ALU.subtract)
                sr = sq_p.tile([1, NHW], F32, tag="sr")
                nc.scalar.activation(sr, var, AF.Sqrt, bias=eps_t[0:1, :], scale=1.0)
                r_bf = sq_p.tile([1, NHW], BF16, tag="r_bf")
                with nc.allow_low_precision(reason="bf16 rstd for bcast matmul"):
                    nc.vector.reciprocal(r_bf, sr)
                mr_bf = sq_p.tile([1, NHW], BF16, tag="mr_bf")
                nc.vector.tensor_tensor(mr_bf, mean, r_bf, ALU.mult)
                rB = bc_p.tile([128, NHW], F32, tag="rB")
                nc.tensor.matmul(rB, onesRow, r_bf, start=True, stop=True)
                mrB = bc_p.tile([128, NHW], F32, tag="mrB")
                nc.tensor.matmul(mrB, onesRow, mr_bf, start=True, stop=True)
                for c in range(lo, hi):
                    eng = nc.vector if (c % 2 == 0) else nc.gpsimd
                    t = tmp_p.tile([128, NHW], F32, tag="xnorm")
                    eng.tensor_tensor(t, f32(xT[:, c, ns]), rB, ALU.mult)
                    dslice = dst[:, c - lo, ns]
                    if gv is not None:
                        t2 = tmp_p.tile([128, NHW], F32, tag="xnorm2")
                        eng.tensor_tensor(t2, t, mrB, ALU.subtract)
                        eng.tensor_scalar(dslice, t2, gv[:, c - lo:c - lo + 1],
                                          bv[:, c - lo:c - lo + 1], ALU.mult, ALU.add)
                    else:
                        eng.tensor_tensor(dslice, t, mrB, ALU.subtract)

        # ---------------- phase B: LN1 (both branches) ----------------
        xgln = core.tile([128, GC, N], FP8, tag="xgln")
        xlln = core.tile([128, GC, N], FP8, tag="xlln")
        with tc.tile_pool(name="sq1", bufs=2) as sq_p, \
             tc.tile_pool(name="tmp1", bufs=2) as tmp_p, \
             tc.tile_pool(name="st1", bufs=1, space="PSUM") as st_p, \
             tc.tile_pool(name="bc1", bufs=2, space="PSUM") as bc_p:
            ln_feat(0, GC, xgln, g1g, g1b, sq_p, st_p, bc_p, tmp_p, "act")
            ln_feat(GC, CC, xlln, l1g, l1b, sq_p, st_p, bc_p, tmp_p, "act")

        # DR contraction helper over GC=3 chunks: pair (0,1) + single 2
        def mm3(ps, w, rhs_t, cols, ns):
            nc.tensor.matmul(ps, w[:, 0:2, cols], rhs_t[:, 0:2, ns],
                             start=True, stop=False, perf_mode=DR)
            nc.tensor.matmul(ps, w[:, 2, cols], rhs_t[:, 2, ns],
                             start=False, stop=True)

        # ---------------- phases C+D: attention (global + dripped local) -----
        qT = core.tile([128, GC, N], FP8, tag="qT")      # x2^6
        kT = core.tile([128, GC, N], FP8, tag="kT")      # x2^6
        vpad = core.tile([128, MC, H * DP], FP8, tag="vpad")  # x2^6, ones col
        oT = core.tile([128, GC, N], FP8, tag="oT")      # x2^6
        qlT = core.tile([128, GC, N], BF16, tag="qlT")   # x2^10
        klT = core.tile([128, GC, N], BF16, tag="klT")   # x2^10
        vlT = core.tile([128, GC, N], FP8, tag="vlT")    # x2^6
        oTl = core.tile([128, GC, N], FP8, tag="oTl")    # x2^6

        with tc.tile_pool(name="esb", bufs=3) as e_p, \
             tc.tile_pool(name="small", bufs=3) as sm_p, \
             tc.tile_pool(name="pqk", bufs=2, space="PSUM") as pq_p, \
             tc.tile_pool(name="psc", bufs=2, space="PSUM") as ps_p, \
             tc.tile_pool(name="po", bufs=1, space="PSUM") as po_p, \
             tc.tile_pool(name="pb", bufs=1, space="PSUM") as pb_p:

            # global q,k (x2^6 after QS rescale of x2^10 psum)
            for mo in range(2 * GC):
                dst = qT if mo < GC else kT
                dc = mo % GC
                for nh in range(NH):
                    ns = slice(nh * NHW, (nh + 1) * NHW)
                    ps = pq_p.tile([128, NHW], F32, tag="pq")
                    mm3(ps, gqk8, xgln, slice(mo * 128, (mo + 1) * 128), ns)
                    nc.vector.tensor_scalar_mul(dst[:, dc, ns], ps, QS)
            # global v (token-major, x2^6), ones col after
            for m in range(MC):
                ps = pq_p.tile([128, NHW], F32, tag="pq")
                psv = ps[:, 0:H * DP]
                nc.tensor.matmul(psv, xgln[:, 0:2, m * 128:(m + 1) * 128],
                                 wv8[:, 0:2, :], start=True, stop=False, perf_mode=DR)
                nc.tensor.matmul(psv, xgln[:, 2, m * 128:(m + 1) * 128],
                                 wv8[:, 2, :], start=False, stop=True)
                nc.gpsimd.tensor_scalar_mul(vpad[:, m, :], psv, QS)
            nc.vector.memset(
                vpad.rearrange("p m (h e) -> p m h e", e=DP)[:, :, :, D:D + 1], 1.0)

            # drip queue: local qkv + banded products, run on scores-loop gaps
            drip_q = []

            def lq_unit(pi, oc, nh):
                def go():
                    ns = slice(nh * NHW, (nh + 1) * NHW)
                    ps = pq_p.tile([128, NHW], F32, tag="pq", name="lq")
                    mm3(ps, lqkv8, xlln,
                        slice(pi * GD + oc * 128, pi * GD + (oc + 1) * 128), ns)
                    if pi == 0:
                        nc.vector.tensor_copy(qlT[:, oc, ns], ps)
                    elif pi == 1:
                        nc.vector.tensor_copy(klT[:, oc, ns], ps)
                    else:
                        nc.gpsimd.tensor_scalar_mul(vlT[:, oc, ns], ps, QS)
                return go

            for pi in range(3):
                for oc in range(GC):
                    for nh in range(NH):
                        drip_q.append(lq_unit(pi, oc, nh))

            prod_m = core.tile([128, GC, N], BF16, tag="prodm")
            prod_0 = core.tile([128, GC, N], BF16, tag="prod0")
            prod_p = core.tile([128, GC, N], BF16, tag="prodp")

            def prod_unit(which):
                def go():
                    if which == 0:
                        nc.vector.memset(prod_m[:, :, 0:1], 0.0)
                        nc.vector.tensor_tensor(prod_m[:, :, 1:N], qlT[:, :, 1:N],
                                                klT[:, :, 0:N - 1], ALU.mult)
                    elif which == 1:
                        nc.vector.tensor_tensor(prod_0, qlT, klT, ALU.mult)
                    else:
                        nc.vector.memset(prod_p[:, :, N - 1:N], 0.0)
                        nc.vector.tensor_tensor(prod_p[:, :, 0:N - 1],
                                                qlT[:, :, 0:N - 1],
                                                klT[:, :, 1:N], ALU.mult)
                return go

            for which in range(3):
                drip_q.append(prod_unit(which))

            def drip(n):
                for _ in range(n):
                    if drip_q:
                        drip_q.pop(0)()

            # scores -> exp -> DoubleRow AV, per (head, n-half)
            for h in range(H):
                hc, hp = h // 2, (h % 2) * 64
                for nh in range(NH):
                    ns = slice(nh * NHW, (nh + 1) * NHW)
                    po = po_p.tile([DP, NHW], F32, tag="po")
                    for mp in range(MC // 2):
                        ps = ps_p.tile([128, 2, NHW], F32, tag="ps")
                        for half in range(2):
                            m = 2 * mp + half
                            nc.tensor.matmul(ps[:, half],
                                             kT[hp:hp + 64, hc, m * 128:(m + 1) * 128],
                                             qT[hp:hp + 64, hc, ns],
                                             start=True, stop=True)
                        e_sb = e_p.tile([128, 2, NHW], FP8, tag="e")
                        nc.scalar.activation(
                            e_sb.rearrange("p a b -> p (a b)"),
                            ps.rearrange("p a b -> p (a b)"), AF.Exp,
                            scale=EXP_SCALE_G)
                        nc.tensor.matmul(po, vpad[:, 2 * mp:2 * mp + 2,
                                                  h * DP:(h + 1) * DP],
                                         e_sb, start=(mp == 0), stop=(mp == MC // 2 - 1),
                                         perf_mode=DR)
                    rcp = sm_p.tile([1, NHW], BF16, tag="rcp")
                    with nc.allow_low_precision(reason="bf16 recip for bcast"):
                        nc.vector.reciprocal(rcp, po[D:D + 1, :])
                    pb = pb_p.tile([64, NHW], F32, tag="pb")
                    nc.tensor.matmul(pb, c1row, rcp, start=True, stop=True)
                    pb_sb = sm_p.tile([64, NHW], BF16, tag="pbsb")
                    nc.gpsimd.tensor_copy(out=pb_sb, in_=pb)
                    nc.vector.tensor_tensor(oT[hp:hp + 64, hc, ns], po[0:D, :],
                                            pb_sb, ALU.mult)
                    drip(2)
            drip(len(drip_q))

        # ---------------- phase E: projections + local attention, nh-major ---
        o_un = core.tile([128, GC, N], BF16, tag="o_un")
        with tc.tile_pool(name="ltmp", bufs=3) as lt_p, \
             tc.tile_pool(name="pesc", bufs=1, space="PSUM") as pe_p, \
             tc.tile_pool(name="pdsum", bufs=1, space="PSUM") as pd_p, \
             tc.tile_pool(name="pab", bufs=2, space="PSUM") as pa_p, \
             tc.tile_pool(name="ppr", bufs=2, space="PSUM") as pp_p:

            def proj(w8, src, dst_row0, bias, mo, ns):
                ps = pp_p.tile([128, NHW], F32, tag="ppr")
                mm3(ps, w8, src, slice(mo * 128, (mo + 1) * 128), ns)
                row = dst_row0 + mo
                if bias is not None:
                    nc.scalar.activation(ps, ps, AF.Identity,
                                         bias=bias[:, mo:mo + 1], scale=DQ_PROJ)
                    nc.vector.tensor_tensor(xT[:, row, ns], f32(xT[:, row, ns]),
                                            ps, ALU.add)
                else:
                    nc.vector.scalar_tensor_tensor(
                        xT[:, row, ns], ps, DQ_PROJ, f32(xT[:, row, ns]),
                        ALU.mult, ALU.add)

            nc.vector.memset(o_un[:, :, 0:1], 0.0)
            for nh in range(NH):
                ns = slice(nh * NHW, (nh + 1) * NHW)
                # global proj + residual into xT rows [0, GD)
                for mo in range(GC):
                    proj(gp8, oT, 0, gpb, mo, ns)
                # head-sums into esc_all [96, 512]: shift si at partition 32*si
                esc_all = pe_p.tile([96, NHW], F32, tag="escall")
                for si, prod in enumerate((prod_m, prod_0, prod_p)):
                    for kc in range(GC):
                        nc.tensor.matmul(esc_all[32 * si:32 * si + H, :],
                                         blkS[:, kc, :], prod[:, kc, ns],
                                         start=(kc == 0), stop=(kc == GC - 1))
                if nh == 0:
                    nc.vector.memset(esc_all[0:H, 0:1], -1e30)
                if nh == NH - 1:
                    nc.vector.memset(esc_all[64:64 + H, NHW - 1:NHW], -1e30)
                ee_all = lt_p.tile([96, NHW], BF16, tag="ee_all")
                nc.vector.memset(ee_all, 0.0)
                for si in range(3):
                    nc.scalar.activation(ee_all[32 * si:32 * si + H, :],
                                         esc_all[32 * si:32 * si + H, :],
                                         AF.Exp, scale=EXP_SCALE_L)
                dsum = pd_p.tile([H, NHW], F32, tag="dsum")
                nc.tensor.matmul(dsum, sumInd, ee_all, start=True, stop=True)
                rr = lt_p.tile([H, NHW], BF16, tag="rr")
                with nc.allow_low_precision(reason="bf16 softmax recip"):
                    nc.vector.reciprocal(rr, dsum)
                # unnormalized o accumulation: eB broadcast via PE, v shifted
                lo_n, hi_n = nh * NHW, (nh + 1) * NHW
                for si in (0, 2, 1):
                    for kc in range(GC):
                        eB = pa_p.tile([128, NHW], F32, tag="eB")
                        nc.tensor.matmul(eB, blkT96[:, si, kc, :], ee_all,
                                         start=True, stop=True)
                        if si == 0:
                            vs, os_, oe = max(lo_n, 1) - 1, max(lo_n, 1), hi_n
                        elif si == 2:
                            vs, os_, oe = lo_n + 1, lo_n, min(hi_n, N - 1)
                        else:
                            vs, os_, oe = lo_n, lo_n, hi_n
                        a_sl = eB[:, os_ - lo_n:oe - lo_n]
                        v_sl = vlT[:, kc, vs:vs + (oe - os_)]
                        eng = nc.gpsimd if si == 2 else nc.vector
                        if si == 0:
                            eng.tensor_tensor(o_un[:, kc, os_:oe], v_sl, a_sl,
                                              ALU.mult)
                        else:
                            t = lt_p.tile([128, NHW], BF16, tag="avt")
                            eng.tensor_tensor(t[:, 0:oe - os_], v_sl, a_sl, ALU.mult)
                            eng.tensor_tensor(o_un[:, kc, os_:oe],
                                              o_un[:, kc, os_:oe],
                                              t[:, 0:oe - os_], ALU.add)
                # normalize at the end: oTl = o_un * broadcast(rr), fp8
                for kc in range(GC):
                    rB = pa_p.tile([128, NHW], F32, tag="eB", name="rB")
                    nc.tensor.matmul(rB, blkT[:, kc, :], rr, start=True, stop=True)
                    nc.gpsimd.tensor_tensor(oTl[:, kc, ns], o_un[:, kc, ns],
                                            rB, ALU.mult)
                # local proj + residual into xT rows [GD, C)
                for mo in range(GC):
                    proj(lp8, oTl, GC, lpb, mo, ns)

        # ---------------- phase F: LN2 ----------------
        hT = core.tile([128, CC, N], FP8, tag="hT")
        with tc.tile_pool(name="sq2", bufs=2) as sq_p, \
             tc.tile_pool(name="tmp2", bufs=2) as tmp_p, \
             tc.tile_pool(name="st2", bufs=1, space="PSUM") as st_p, \
             tc.tile_pool(name="bc2", bufs=2, space="PSUM") as bc_p:
            ln_feat(0, CC, hT, g2g, g2b, sq_p, st_p, bc_p, tmp_p, "pool")

        # ---------------- phase G: MLP + transpose out ----------------
        with tc.tile_pool(name="gl", bufs=1) as gl_pool, \
             tc.tile_pool(name="otok", bufs=2) as otok_p, \
             tc.tile_pool(name="outT", bufs=1) as outT_p, \
             tc.tile_pool(name="pm", bufs=2, space="PSUM") as pm_p, \
             tc.tile_pool(name="pz", bufs=2, space="PSUM") as pz_p, \
             tc.tile_pool(name="ps_tr3", bufs=2, space="PSUM") as ps_tr3:
            gls = [gl_pool.tile([128, 2, NHW], FP8, tag=f"gl{jp}", name=f"gl{jp}")
                   for jp in range(JC // 2)]
            for nh in range(NH):
                ns = slice(nh * NHW, (nh + 1) * NHW)
                outT = outT_p.tile([128, CC, NHW], F32, tag="outT")
                # fc1 (hi+lo DR) -> gelu(pair) -> gl fp8
                for jp in range(JC // 2):
                    pm = pm_p.tile([128, 2, NHW], F32, tag="pm")
                    for half in range(2):
                        j = 2 * jp + half
                        js = slice(j * 128, (j + 1) * 128)
                        for t in range(CC // 2):
                            nc.tensor.matmul(pm[:, half], fc1h[:, 2 * t:2 * t + 2, js],
                                             hT[:, 2 * t:2 * t + 2, ns],
                                             start=(t == 0), stop=False, perf_mode=DR)
                        for t in range(CC // 2):
                            nc.tensor.matmul(pm[:, half], fc1l[:, 2 * t:2 * t + 2, js],
                                             hT[:, 2 * t:2 * t + 2, ns],
                                             start=False, stop=(t == CC // 2 - 1),
                                             perf_mode=DR)
                    gl = gls[jp]
                    if fc1b is not None:
                        # bias is per hidden unit = per psum partition, halves differ
                        for half in range(2):
                            j = 2 * jp + half
                            nc.scalar.activation(gl[:, half], pm[:, half], AF.Gelu,
                                                 bias=fc1b[:, j:j + 1], scale=DQ_FC)
                    else:
                        nc.scalar.activation(gl.rearrange("p a b -> p (a b)"),
                                             pm.rearrange("p a b -> p (a b)"),
                                             AF.Gelu, scale=DQ_FC)
                # fc2 (hi+lo DR) per output chunk, then residual + transpose
                for mo in range(CC):
                    cs = slice(mo * 128, (mo + 1) * 128)
                    zp = pz_p.tile([128, NHW], F32, tag="pz")
                    for jp in range(JC // 2):
                        nc.tensor.matmul(zp, fc2h[:, 2 * jp:2 * jp + 2, cs], gls[jp],
                                         start=(jp == 0), stop=False, perf_mode=DR)
                    for jp in range(JC // 2):
                        nc.tensor.matmul(zp, fc2l[:, 2 * jp:2 * jp + 2, cs], gls[jp],
                                         start=False, stop=(jp == JC // 2 - 1),
                                         perf_mode=DR)
                    if fc2b is not None:
                        nc.scalar.activation(zp, zp, AF.Identity,
                                             bias=fc2b[:, mo:mo + 1], scale=DQ_FC)
                        nc.vector.tensor_tensor(outT[:, mo], f32(xT[:, mo, ns]),
                                                zp, ALU.add)
                    else:
                        nc.vector.scalar_tensor_tensor(
                            outT[:, mo], zp, DQ_FC, f32(xT[:, mo, ns]),
                            ALU.mult, ALU.add)
                # transpose out + store (4 m-chunks per half)
                outTr = outT.bitcast(F32R)
                for mq in range(NHW // 128):
                    ot = otok_p.tile([128, C], F32, tag="ot")
                    for cq in range(CC // 2):
                        ps = ps_tr3.tile([128, 2, 128], F32R, tag="tr3")
                        for half in range(2):
                            c = 2 * cq + half
                            nc.tensor.transpose(ps[:, half],
                                                outTr[:, c, mq * 128:(mq + 1) * 128],
                                                identR)
                        dst = ot[:, 2 * cq * 128:(2 * cq + 2) * 128]
                        dst = dst.rearrange("p (a b) -> p a b", a=2)
                        if (mq + cq) % 2 == 0:
                            nc.gpsimd.tensor_copy(out=dst, in_=f32(ps))
                        else:
                            nc.scalar.copy(dst, f32(ps))
                    tok0 = nh * NHW + mq * 128
                    nc.sync.dma_start(out_d[tok0:tok0 + 128, :], ot)

    nc.compile()
    return nc


_NC_CACHE = {}


def _q8(w, s=WS):
    return np.clip(w.astype(np.float64) * s, -240.0, 240.0).astype(E4NP)


def _q8_split(w, s=WS):
    ws = np.clip(w.astype(np.float64) * s, -240.0, 240.0)
    hi = ws.astype(E4NP)
    lo = np.clip(ws - hi.astype(np.float64), -240.0, 240.0).astype(E4NP)
    return hi, lo


def kernel(**inputs):
    inp = {k: np.ascontiguousarray(np.asarray(v), dtype=np.float32)
           for k, v in inputs.items()}
    flags = {
        "gb1g": not (np.all(inp["ln1_g"] == 1.0) and np.all(inp["ln1_b"] == 0.0)),
        "gb1l": not (np.all(inp["ln1l_g"] == 1.0) and np.all(inp["ln1l_b"] == 0.0)),
        "gb2": not (np.all(inp["ln2_g"] == 1.0) and np.all(inp["ln2_b"] == 0.0)),
        "bias_gproj": bool(np.any(inp["g_proj_b"] != 0.0)),
        "bias_lproj": bool(np.any(inp["l_proj_b"] != 0.0)),
        "bias_fc1": bool(np.any(inp["fc1_b"] != 0.0)),
        "bias_fc2": bool(np.any(inp["fc2_b"] != 0.0)),
    }
    key = tuple(sorted(flags.items()))
    nc = _NC_CACHE.get(key)
    if nc is None:
        nc = _build(flags)
        _NC_CACHE[key] = nc

    g_qkv = inp["g_qkv_w"]
    wv = np.zeros((GD, H * DP), np.float32)
    wv.reshape(GD, H, DP)[:, :, :D] = g_qkv[:, 2 * GD:].reshape(GD, H, D)
    fc1h, fc1l = _q8_split(inp["fc1_w"])
    fc2h, fc2l = _q8_split(inp["fc2_w"])
    weights = {
        "gqk8": _q8(g_qkv[:, :2 * GD]),
        "wv8": _q8(wv),
        "lqkv8": _q8(inp["l_qkv_w"]),
        "gp8": _q8(inp["g_proj_w"]),
        "lp8": _q8(inp["l_proj_w"]),
        "fc1h": fc1h, "fc1l": fc1l, "fc2h": fc2h, "fc2l": fc2l,
    }
    for nm, fl in (("ln1_g", "gb1g"), ("ln1_b", "gb1g"), ("ln1l_g", "gb1l"),
                   ("ln1l_b", "gb1l"), ("ln2_g", "gb2"), ("ln2_b", "gb2"),
                   ("g_proj_b", "bias_gproj"), ("l_proj_b", "bias_lproj"),
                   ("fc1_b", "bias_fc1"), ("fc2_b", "bias_fc2")):
        if flags[fl]:
            weights[nm] = inp[nm]

    x = inp["x"]
    in_maps = [dict(weights, x=np.ascontiguousarray(x[b])) for b in range(B)]
    res = run_bass_kernel_spmd(nc, in_maps, core_ids=list(range(B)))
    return np.stack([res.results[b]["out"] for b in range(B)]).astype(np.float32)


# revision 20
# speedup vs baseline: 1.6086x; 1.0938x over previous
"""Trainium2 Bass kernel for nn_Block_local (dual global/banded-local attention block).

Sharding: data-parallel, one batch element per NeuronCore (B=8, 8 cores).
Feature-major activations ([C,N]); fp8e4 DoubleRow matmuls for all
weight-contractions (weights quantized host-side, fc1/fc2 split hi+lo fp8),
bf16 scores, fp8 softmax/activation intermediates, feature-major banded local
attention (shifts are free-axis slices; no shift DMAs, no local transposes).
"""
import os
import numpy as np
import ml_dtypes

import concourse.bass as bass
import concourse.bacc as bacc
import concourse.mybir as mybir
import concourse.tile as tile
from concourse.bass_utils import run_bass_kernel_spmd
from concourse.masks import make_identity
from contextlib import ExitStack

F32 = mybir.dt.float32
F32R = mybir.dt.float32r
BF16 = mybir.dt.bfloat16
FP8 = mybir.dt.float8e4
AF = mybir.ActivationFunctionType
ALU = mybir.AluOpType
DR = mybir.MatmulPerfMode.DoubleRow
E4NP = ml_dtypes.float8_e4m3

B, N, C = 8, 1024, 768
GD = 384
H, D = 6, 64
DP = D + 1              # v head dim padded with ones column
SCALE = D ** -0.5
HID = 3072
EPS = 1e-6
NH = 2                  # token n-halves of 512
NHW = N // NH           # 512
MC = N // 128           # 8 token chunks
CC = C // 128           # 6 feature chunks
GC = GD // 128          # 3 feature chunks per branch
JC = HID // 128         # 24 hidden chunks
WS = 1024.0             # weight quant scale (2^10)
QS = 2.0 ** -4          # q/k/v psum -> fp8 rescale (carries 2^6)
DQ_PROJ = 2.0 ** -16    # proj psum dequant (oT 2^6 * W 2^10)
DQ_FC = 2.0 ** -10      # fc psum dequant (acts true-scale, W 2^10)
EXP_SCALE_G = SCALE * 2.0 ** -12  # global: q,k each carry 2^6
EXP_SCALE_L = SCALE * 2.0 ** -20  # local: ql,kl each carry 2^10


def f32(ap):
    return ap.bitcast(F32)


def _build(flags):
    nc = bacc.Bacc("TRN2", target_bir_lowering=False, debug=False)

    x_d = nc.dram_tensor("x", (N, C), F32, kind="ExternalInput")
    gqk8_d = nc.dram_tensor("gqk8", (GD, 2 * GD), FP8, kind="ExternalInput")
    wv8_d = nc.dram_tensor("wv8", (GD, H * DP), FP8, kind="ExternalInput")
    lqkv8_d = nc.dram_tensor("lqkv8", (GD, 3 * GD), FP8, kind="ExternalInput")
    gp8_d = nc.dram_tensor("gp8", (GD, GD), FP8, kind="ExternalInput")
    lp8_d = nc.dram_tensor("lp8", (GD, GD), FP8, kind="ExternalInput")
    fc1h_d = nc.dram_tensor("fc1h", (C, HID), FP8, kind="ExternalInput")
    fc1l_d = nc.dram_tensor("fc1l", (C, HID), FP8, kind="ExternalInput")
    fc2h_d = nc.dram_tensor("fc2h", (HID, C), FP8, kind="ExternalInput")
    fc2l_d = nc.dram_tensor("fc2l", (HID, C), FP8, kind="ExternalInput")
    opt = {}
    for nm, sz, fl in (("ln1_g", GD, "gb1g"), ("ln1_b", GD, "gb1g"),
                       ("ln1l_g", GD, "gb1l"), ("ln1l_b", GD, "gb1l"),
                       ("ln2_g", C, "gb2"), ("ln2_b", C, "gb2"),
                       ("g_proj_b", GD, "bias_gproj"), ("l_proj_b", GD, "bias_lproj"),
                       ("fc1_b", HID, "bias_fc1"), ("fc2_b", C, "bias_fc2")):
        if flags[fl]:
            opt[nm] = nc.dram_tensor(nm, (sz,), F32, kind="ExternalInput")
    out_d = nc.dram_tensor("out", (N, C), F32, kind="ExternalOutput")

    gqk8_v = gqk8_d.rearrange("(kc p) c -> p kc c", p=128)
    wv8_v = wv8_d.rearrange("(kc p) c -> p kc c", p=128)
    lqkv8_v = lqkv8_d.rearrange("(kc p) c -> p kc c", p=128)
    gp8_v = gp8_d.rearrange("(kc p) c -> p kc c", p=128)
    lp8_v = lp8_d.rearrange("(kc p) c -> p kc c", p=128)
    fc1h_v = fc1h_d.rearrange("(kc p) c -> p kc c", p=128)
    fc1l_v = fc1l_d.rearrange("(kc p) c -> p kc c", p=128)
    fc2h_v = fc2h_d.rearrange("(kc p) c -> p kc c", p=128)
    fc2l_v = fc2l_d.rearrange("(kc p) c -> p kc c", p=128)

    with tile.TileContext(nc) as tc, ExitStack() as top:
        consts = top.enter_context(tc.tile_pool(name="consts", bufs=1))
        core = top.enter_context(tc.tile_pool(name="core", bufs=1))
        wpool = top.enter_context(tc.tile_pool(name="wpool", bufs=1))

        identF = consts.tile([128, 128], F32, tag="identF")
        make_identity(nc, identF)
        identR = identF.bitcast(F32R)
        onesR = consts.tile([128, 1], F32, tag="onesR")
        nc.vector.memset(onesR, 1.0)
        onesR = onesR.bitcast(F32R)
        onesB2 = consts.tile([128, 1], BF16, tag="onesB2")
        nc.vector.memset(onesB2, 1.0)
        onesRow = consts.tile([1, 128], BF16, tag="onesRow")
        nc.vector.memset(onesRow, 1.0)
        c1row = consts.tile([1, 64], BF16, tag="c1row")
        nc.vector.memset(c1row, 1.0)
        eps_t = consts.tile([1, 1], F32, tag="eps")
        nc.vector.memset(eps_t, EPS)
        # blkS[p, kc, j]: headsum lhsT (1 if j == 2*kc + p//64)
        blkS = consts.tile([128, GC, 2 * GC], BF16, tag="blkS")
        nc.vector.memset(blkS, 0.0)
        for kc in range(GC):
            nc.vector.memset(blkS[0:64, kc, 2 * kc:2 * kc + 1], 1.0)
            nc.vector.memset(blkS[64:128, kc, 2 * kc + 1:2 * kc + 2], 1.0)
        # blkT[j, kc, p]: head->partition broadcast lhsT (j = head index 0..5)
        blkT = consts.tile([H, GC, 128], BF16, tag="blkT")
        nc.vector.memset(blkT, 0.0)
        for kc in range(GC):
            nc.vector.memset(blkT[2 * kc:2 * kc + 1, kc, 0:64], 1.0)
            nc.vector.memset(blkT[2 * kc + 1:2 * kc + 2, kc, 64:128], 1.0)
        # blkT96[32*si + j, si, kc, p]: per-shift head->partition broadcast
        blkT96 = consts.tile([96, 3, GC, 128], BF16, tag="blkT96")
        nc.vector.memset(blkT96, 0.0)
        for si in range(3):
            for kc in range(GC):
                r0 = 32 * si + 2 * kc
                nc.vector.memset(blkT96[r0:r0 + 1, si, kc, 0:64], 1.0)
                nc.vector.memset(blkT96[r0 + 1:r0 + 2, si, kc, 64:128], 1.0)
        # sumInd[32*si + j, j] = 1: sums the 3 shift groups per head
        sumInd = consts.tile([96, H], BF16, tag="sumInd")
        nc.vector.memset(sumInd, 0.0)
        for si in range(3):
            for j in range(H):
                nc.vector.memset(sumInd[32 * si + j:32 * si + j + 1, j:j + 1], 1.0)

        def load_vec(dram, n_elems, tag):
            t = consts.tile([128, n_elems // 128], F32, tag=tag)
            nc.sync.dma_start(t, dram.rearrange("(c p) -> p c", p=128))
            return t

        g1g = load_vec(opt["ln1_g"], GD, "g1g") if flags["gb1g"] else None
        g1b = load_vec(opt["ln1_b"], GD, "g1b") if flags["gb1g"] else None
        l1g = load_vec(opt["ln1l_g"], GD, "l1g") if flags["gb1l"] else None
        l1b = load_vec(opt["ln1l_b"], GD, "l1b") if flags["gb1l"] else None
        g2g = load_vec(opt["ln2_g"], C, "g2g") if flags["gb2"] else None
        g2b = load_vec(opt["ln2_b"], C, "g2b") if flags["gb2"] else None
        gpb = load_vec(opt["g_proj_b"], GD, "gpb") if flags["bias_gproj"] else None
        lpb = load_vec(opt["l_proj_b"], GD, "lpb") if flags["bias_lproj"] else None
        fc1b = load_vec(opt["fc1_b"], HID, "fc1b") if flags["bias_fc1"] else None
        fc2b = load_vec(opt["fc2_b"], C, "fc2b") if flags["bias_fc2"] else None

        # resident fp8 weights (DMA'd on the sync queue AFTER x, before use)
        gqk8 = wpool.tile([128, GC, 2 * GD], FP8, tag="gqk8")
        wv8 = wpool.tile([128, GC, H * DP], FP8, tag="wv8")
        lqkv8 = wpool.tile([128, GC, 3 * GD], FP8, tag="lqkv8")
        gp8 = wpool.tile([128, GC, GD], FP8, tag="gp8")
        lp8 = wpool.tile([128, GC, GD], FP8, tag="lp8")
        fc1h = wpool.tile([128, CC, HID], FP8, tag="fc1h")
        fc1l = wpool.tile([128, CC, HID], FP8, tag="fc1l")
        fc2h = wpool.tile([128, JC, C], FP8, tag="fc2h")
        fc2l = wpool.tile([128, JC, C], FP8, tag="fc2l")

        def dma_weights():
            nc.sync.dma_start(gqk8, gqk8_v)
            nc.sync.dma_start(wv8, wv8_v)
            nc.sync.dma_start(lqkv8, lqkv8_v)
            nc.sync.dma_start(gp8, gp8_v)
            nc.sync.dma_start(lp8, lp8_v)
            for kc in range(0, CC, 2):
                nc.sync.dma_start(fc1h[:, kc:kc + 2], fc1h_v[:, kc:kc + 2])
                nc.sync.dma_start(fc1l[:, kc:kc + 2], fc1l_v[:, kc:kc + 2])
            for kc in range(0, JC, 8):
                nc.sync.dma_start(fc2h[:, kc:kc + 8], fc2h_v[:, kc:kc + 8])
                nc.sync.dma_start(fc2l[:, kc:kc + 8], fc2l_v[:, kc:kc + 8])

        xT = core.tile([128, CC, N], F32R, tag="xT")   # residual, feature-major

        # ---------------- phase A: load x, transpose to feature-major --------
        x_v = x_d.rearrange("(mq two p) c -> mq p two c", p=128, two=2)
        with tc.tile_pool(name="xtok", bufs=4) as xtok_p, \
             tc.tile_pool(name="ps_tr0", bufs=3, space="PSUM") as ps_tr0:
            xts = []
            for mq in range(MC // 2):
                xt = xtok_p.tile([128, 2, C], F32, tag="xt", name=f"xt{mq}")
                nc.sync.dma_start(xt, x_v[mq])
                xts.append(xt)
            dma_weights()
            for mq in range(MC // 2):
                xtr = xts[mq].bitcast(F32R)
                for half in range(2):
                    m = 2 * mq + half
                    for cq in range(CC // 2):
                        ps = ps_tr0.tile([128, 2, 128], F32R, tag="tr")
                        for h2 in range(2):
                            c = 2 * cq + h2
                            nc.tensor.transpose(
                                ps[:, h2], xtr[:, half, c * 128:(c + 1) * 128], identR)
                        dst = xT[:, 2 * cq:2 * cq + 2, m * 128:(m + 1) * 128]
                        if (m + cq) % 2 == 0:
                            nc.vector.tensor_copy(f32(dst), f32(ps))
                        else:
                            nc.scalar.copy(f32(dst), f32(ps))

        # ---------------- feature-major LayerNorm helper ----------------
        def ln_feat(lo, hi, dst, gv, bv, sq_p, st_p, bc_p, tmp_p, sq_eng):
            """dst[:, c-lo, :] = fp8(LN(xT rows [lo*128, hi*128)) over features)."""
            nch = hi - lo
            inv = 1.0 / (nch * 128)
            for nh in range(NH):
                ns = slice(nh * NHW, (nh + 1) * NHW)
                st = st_p.tile([1, 2 * NHW], F32, tag="stat")
                for i, c in enumerate(range(lo, hi)):
                    nc.tensor.matmul(st[:, 0:NHW], onesR[:, 0:1], xT[:, c, ns],
                                     start=(i == 0), stop=(i == nch - 1))
                for i, c in enumerate(range(lo, hi)):
                    sq = sq_p.tile([128, NHW], BF16, tag="sq")
                    if sq_eng == "act":
                        nc.scalar.activation(sq, f32(xT[:, c, ns]), AF.Square)
                    else:
                        nc.gpsimd.tensor_tensor(sq, f32(xT[:, c, ns]),
                                                f32(xT[:, c, ns]), ALU.mult)
                    nc.tensor.matmul(st[:, NHW:2 * NHW], onesB2[:, 0:1], sq,
                                     start=(i == 0), stop=(i == nch - 1))
                mean = sq_p.tile([1, NHW], F32, tag="mean")
                nc.vector.tensor_scalar_mul(mean, st[:, 0:NHW], inv)
                e2 = sq_p.tile([1, NHW], F32, tag="e2")
                nc.vector.tensor_scalar_mul(e2, st[:, NHW:2 * NHW], inv)
                var = sq_p.tile([1, NHW], F32, tag="var")
                nc.vector.tensor_tensor(var, mean, mean, ALU.mult)
                nc.vector.tensor_tensor(var, e2, var, ALU.subtract)
                sr = sq_p.tile([1, NHW], F32, tag="sr")
                nc.scalar.activation(sr, var, AF.Sqrt, bias=eps_t[0:1, :], scale=1.0)
                r_bf = sq_p.tile([1, NHW], BF16, tag="r_bf")
                with nc.allow_low_precision(reason="bf16 rstd for bcast matmul"):
                    nc.vector.reciprocal(r_bf, sr)
                mr_bf = sq_p.tile([1, NHW], BF16, tag="mr_bf")
                nc.vector.tensor_tensor(mr_bf, mean, r_bf, ALU.mult)
                rB = bc_p.tile([128, NHW], F32, tag="rB")
                nc.tensor.matmul(rB, onesRow, r_bf, start=True, stop=True)
                mrB = bc_p.tile([128, NHW], F32, tag="mrB")
                nc.tensor.matmul(mrB, onesRow, mr_bf, start=True, stop=True)
                for c in range(lo, hi):
                    eng = nc.vector if (c % 2 == 0) else nc.gpsimd
                    t = tmp_p.tile([128, NHW], F32, tag="xnorm")
                    eng.tensor_tensor(t, f32(xT[:, c, ns]), rB, ALU.mult)
                    dslice = dst[:, c - lo, ns]
                    if gv is not None:
                        t2 = tmp_p.tile([128, NHW], F32, tag="xnorm2")
                        eng.tensor_tensor(t2, t, mrB, ALU.subtract)
                        eng.tensor_scalar(dslice, t2, gv[:, c - lo:c - lo + 1],
                                          bv[:, c - lo:c - lo + 1], ALU.mult, ALU.add)
                    else:
                        eng.tensor_tensor(dslice, t, mrB, ALU.subtract)

        # ---------------- phase B: LN1 (both branches) ----------------
        xgln = core.tile([128, GC, N], FP8, tag="xgln")
        xlln = core.tile([128, GC, N], FP8, tag="xlln")
        with tc.tile_pool(name="sq1", bufs=2) as sq_p, \
             tc.tile_pool(name="tmp1", bufs=2) as tmp_p, \
             tc.tile_pool(name="st1", bufs=1, space="PSUM") as st_p, \
             tc.tile_pool(name="bc1", bufs=2, space="PSUM") as bc_p:
            ln_feat(0, GC, xgln, g1g, g1b, sq_p, st_p, bc_p, tmp_p, "act")
            ln_feat(GC, CC, xlln, l1g, l1b, sq_p, st_p, bc_p, tmp_p, "act")

        # DR contraction helper over GC=3 chunks: pair (0,1) + single 2
        def mm3(ps, w, rhs_t, cols, ns):
            nc.tensor.matmul(ps, w[:, 0:2, cols], rhs_t[:, 0:2, ns],
                             start=True, stop=False, perf_mode=DR)
            nc.tensor.matmul(ps, w[:, 2, cols], rhs_t[:, 2, ns],
                             start=False, stop=True)

        # ---------------- phases C+D: attention (global + dripped local) -----
        qT = core.tile([128, GC, N], FP8, tag="qT")      # x2^6
        kT = core.tile([128, GC, N], FP8, tag="kT")      # x2^6
        vpad = core.tile([128, MC, H * DP], FP8, tag="vpad")  # x2^6, ones col
        oT = core.tile([128, GC, N], FP8, tag="oT")      # x2^6
        qlT = core.tile([128, GC, N], BF16, tag="qlT")   # x2^10
        klT = core.tile([128, GC, N], BF16, tag="klT")   # x2^10
        vlT = core.tile([128, GC, N], FP8, tag="vlT")    # x2^6
        oTl = core.tile([128, GC, N], FP8, tag="oTl")    # x2^6

        with tc.tile_pool(name="esb", bufs=3) as e_p, \
             tc.tile_pool(name="small", bufs=3) as sm_p, \
             tc.tile_pool(name="pqk", bufs=2, space="PSUM") as pq_p, \
             tc.tile_pool(name="psc", bufs=2, space="PSUM") as ps_p, \
             tc.tile_pool(name="po", bufs=1, space="PSUM") as po_p, \
             tc.tile_pool(name="pb", bufs=1, space="PSUM") as pb_p:

            # global q,k (x2^6 after QS rescale of x2^10 psum)
            for mo in range(2 * GC):
                dst = qT if mo < GC else kT
                dc = mo % GC
                for nh in range(NH):
                    ns = slice(nh * NHW, (nh + 1) * NHW)
                    ps = pq_p.tile([128, NHW], F32, tag="pq")
                    mm3(ps, gqk8, xgln, slice(mo * 128, (mo + 1) * 128), ns)
                    nc.vector.tensor_scalar_mul(dst[:, dc, ns], ps, QS)
            # global v (token-major, x2^6), ones col after
            for m in range(MC):
                ps = pq_p.tile([128, NHW], F32, tag="pq")
                psv = ps[:, 0:H * DP]
                nc.tensor.matmul(psv, xgln[:, 0:2, m * 128:(m + 1) * 128],
                                 wv8[:, 0:2, :], start=True, stop=False, perf_mode=DR)
                nc.tensor.matmul(psv, xgln[:, 2, m * 128:(m + 1) * 128],
                                 wv8[:, 2, :], start=False, stop=True)
                nc.gpsimd.tensor_scalar_mul(vpad[:, m, :], psv, QS)
            nc.vector.memset(
                vpad.rearrange("p m (h e) -> p m h e", e=DP)[:, :, :, D:D + 1], 1.0)

            # drip queue: local qkv + banded products, run on scores-loop gaps
            drip_q = []

            def lq_unit(pi, oc, nh):
                def go():
                    ns = slice(nh * NHW, (nh + 1) * NHW)
                    ps = pq_p.tile([128, NHW], F32, tag="pq", name="lq")
                    mm3(ps, lqkv8, xlln,
                        slice(pi * GD + oc * 128, pi * GD + (oc + 1) * 128), ns)
                    if pi == 0:
                        nc.vector.tensor_copy(qlT[:, oc, ns], ps)
                    elif pi == 1:
                        nc.vector.tensor_copy(klT[:, oc, ns], ps)
                    else:
                        nc.gpsimd.tensor_scalar_mul(vlT[:, oc, ns], ps, QS)
                return go

            for pi in (1, 0, 2):
                for oc in range(GC):
                    for nh in range(NH):
                        drip_q.append(lq_unit(pi, oc, nh))

            prod_m = core.tile([128, GC, N], BF16, tag="prodm")
            prod_0 = core.tile([128, GC, N], BF16, tag="prod0")
            prod_p = core.tile([128, GC, N], BF16, tag="prodp")

            def prod_unit(which):
                def go():
                    if which == 0:
                        nc.vector.memset(prod_m[:, :, 0:1], 0.0)
                        nc.vector.tensor_tensor(prod_m[:, :, 1:N], qlT[:, :, 1:N],
                                                klT[:, :, 0:N - 1], ALU.mult)
                    elif which == 1:
                        nc.vector.tensor_tensor(prod_0, qlT, klT, ALU.mult)
                    else:
                        nc.vector.memset(prod_p[:, :, N - 1:N], 0.0)
                        nc.vector.tensor_tensor(prod_p[:, :, 0:N - 1],
                                                qlT[:, :, 0:N - 1],
                                                klT[:, :, 1:N], ALU.mult)
                return go

            for which in range(3):
                drip_q.append(prod_unit(which))

            def drip(n):
                for _ in range(n):
                    if drip_q:
                        drip_q.pop(0)()

            # scores -> exp -> DoubleRow AV, per (head, n-half)
            for h in range(H):
                hc, hp = h // 2, (h % 2) * 64
                for nh in range(NH):
                    ns = slice(nh * NHW, (nh + 1) * NHW)
                    po = po_p.tile([DP, NHW], F32, tag="po")
                    for mp in range(MC // 2):
                        ps = ps_p.tile([128, 2, NHW], F32, tag="ps")
                        for half in range(2):
                            m = 2 * mp + half
                            nc.tensor.matmul(ps[:, half],
                                             kT[hp:hp + 64, hc, m * 128:(m + 1) * 128],
                                             qT[hp:hp + 64, hc, ns],
                                             start=True, stop=True)
                        e_sb = e_p.tile([128, 2, NHW], FP8, tag="e")
                        nc.scalar.activation(
                            e_sb.rearrange("p a b -> p (a b)"),
                            ps.rearrange("p a b -> p (a b)"), AF.Exp,
                            scale=EXP_SCALE_G)
                        nc.tensor.matmul(po, vpad[:, 2 * mp:2 * mp + 2,
                                                  h * DP:(h + 1) * DP],
                                         e_sb, start=(mp == 0), stop=(mp == MC // 2 - 1),
                                         perf_mode=DR)
                    rcp = sm_p.tile([1, NHW], BF16, tag="rcp")
                    with nc.allow_low_precision(reason="bf16 recip for bcast"):
                        nc.vector.reciprocal(rcp, po[D:D + 1, :])
                    pb = pb_p.tile([64, NHW], F32, tag="pb")
                    nc.tensor.matmul(pb, c1row, rcp, start=True, stop=True)
                    pb_sb = sm_p.tile([64, NHW], BF16, tag="pbsb")
                    nc.gpsimd.tensor_copy(out=pb_sb, in_=pb)
                    nc.vector.tensor_tensor(oT[hp:hp + 64, hc, ns], po[0:D, :],
                                            pb_sb, ALU.mult)
                    drip(2)
            drip(len(drip_q))

        # ---------------- phase E: projections + local attention, nh-major ---
        o_un = core.tile([128, GC, N], BF16, tag="o_un")
        with tc.tile_pool(name="ltmp", bufs=3) as lt_p, \
             tc.tile_pool(name="pesc", bufs=1, space="PSUM") as pe_p, \
             tc.tile_pool(name="pdsum", bufs=1, space="PSUM") as pd_p, \
             tc.tile_pool(name="pab", bufs=2, space="PSUM") as pa_p, \
             tc.tile_pool(name="ppr", bufs=2, space="PSUM") as pp_p:

            def proj(w8, src, dst_row0, bias, mo, ns):
                ps = pp_p.tile([128, NHW], F32, tag="ppr")
                mm3(ps, w8, src, slice(mo * 128, (mo + 1) * 128), ns)
                row = dst_row0 + mo
                if bias is not None:
                    nc.scalar.activation(ps, ps, AF.Identity,
                                         bias=bias[:, mo:mo + 1], scale=DQ_PROJ)
                    nc.vector.tensor_tensor(xT[:, row, ns], f32(xT[:, row, ns]),
                                            ps, ALU.add)
                else:
                    nc.vector.scalar_tensor_tensor(
                        xT[:, row, ns], ps, DQ_PROJ, f32(xT[:, row, ns]),
                        ALU.mult, ALU.add)

            nc.vector.memset(o_un[:, :, 0:1], 0.0)
            for nh in range(NH):
                ns = slice(nh * NHW, (nh + 1) * NHW)
                # global proj + residual into xT rows [0, GD)
                for mo in range(GC):
                    proj(gp8, oT, 0, gpb, mo, ns)
                # head-sums into esc_all [96, 512]: shift si at partition 32*si
                esc_all = pe_p.tile([96, NHW], F32, tag="escall")
                for si, prod in enumerate((prod_m, prod_0, prod_p)):
                    for kc in range(GC):
                        nc.tensor.matmul(esc_all[32 * si:32 * si + H, :],
                                         blkS[:, kc, :], prod[:, kc, ns],
                                         start=(kc == 0), stop=(kc == GC - 1))
                if nh == 0:
                    nc.vector.memset(esc_all[0:H, 0:1], -1e30)
                if nh == NH - 1:
                    nc.vector.memset(esc_all[64:64 + H, NHW - 1:NHW], -1e30)
                ee_all = lt_p.tile([96, NHW], BF16, tag="ee_all")
                nc.vector.memset(ee_all, 0.0)
                for si in range(3):
                    nc.scalar.activation(ee_all[32 * si:32 * si + H, :],
                                         esc_all[32 * si:32 * si + H, :],
                                         AF.Exp, scale=EXP_SCALE_L)
                dsum = pd_p.tile([H, NHW], F32, tag="dsum")
                nc.tensor.matmul(dsum, sumInd, ee_all, start=True, stop=True)
                rr = lt_p.tile([H, NHW], BF16, tag="rr")
                with nc.allow_low_precision(reason="bf16 softmax recip"):
                    nc.vector.reciprocal(rr, dsum)
                # unnormalized o accumulation: eB broadcast via PE, v shifted
                lo_n, hi_n = nh * NHW, (nh + 1) * NHW
                for si in (0, 2, 1):
                    for kc in range(GC):
                        eB = pa_p.tile([128, NHW], F32, tag="eB")
                        nc.tensor.matmul(eB, blkT96[:, si, kc, :], ee_all,
                                         start=True, stop=True)
                        if si == 0:
                            vs, os_, oe = max(lo_n, 1) - 1, max(lo_n, 1), hi_n
                        elif si == 2:
                            vs, os_, oe = lo_n + 1, lo_n, min(hi_n, N - 1)
                        else:
                            vs, os_, oe = lo_n, lo_n, hi_n
                        a_sl = eB[:, os_ - lo_n:oe - lo_n]
                        v_sl = vlT[:, kc, vs:vs + (oe - os_)]
                        eng = nc.vector
                        if si == 0:
                            eng.tensor_tensor(o_un[:, kc, os_:oe], v_sl, a_sl,
                                              ALU.mult)
                        else:
                            t = lt_p.tile([128, NHW], BF16, tag="avt")
                            eng.tensor_tensor(t[:, 0:oe - os_], v_sl, a_sl, ALU.mult)
                            eng.tensor_tensor(o_un[:, kc, os_:oe],
                                              o_un[:, kc, os_:oe],
                                              t[:, 0:oe - os_], ALU.add)
                # normalize at the end: oTl = o_un * broadcast(rr), fp8
                for kc in range(GC):
                    rB = pa_p.tile([128, NHW], F32, tag="eB", name="rB")
                    nc.tensor.matmul(rB, blkT[:, kc, :], rr, start=True, stop=True)
                    nc.gpsimd.tensor_tensor(oTl[:, kc, ns], o_un[:, kc, ns],
                                            rB, ALU.mult)
                # local proj + residual into xT rows [GD, C)
                for mo in range(GC):
                    proj(lp8, oTl, GC, lpb, mo, ns)

        # ---------------- phase F: LN2 ----------------
        hT = core.tile([128, CC, N], FP8, tag="hT")
        with tc.tile_pool(name="sq2", bufs=2) as sq_p, \
             tc.tile_pool(name="tmp2", bufs=2) as tmp_p, \
             tc.tile_pool(name="st2", bufs=1, space="PSUM") as st_p, \
             tc.tile_pool(name="bc2", bufs=2, space="PSUM") as bc_p:
            ln_feat(0, CC, hT, g2g, g2b, sq_p, st_p, bc_p, tmp_p, "act")

        # ---------------- phase G: MLP + transpose out ----------------
        with tc.tile_pool(name="gl", bufs=1) as gl_pool, \
             tc.tile_pool(name="otok", bufs=2) as otok_p, \
             tc.tile_pool(name="outT", bufs=1) as outT_p, \
             tc.tile_pool(name="pm", bufs=2, space="PSUM") as pm_p, \
             tc.tile_pool(name="pz", bufs=2, space="PSUM") as pz_p, \
             tc.tile_pool(name="ps_tr3", bufs=2, space="PSUM") as ps_tr3:
            gls = [gl_pool.tile([128, 2, NHW], FP8, tag=f"gl{jp}", name=f"gl{jp}")
                   for jp in range(JC // 2)]
            for nh in range(NH):
                ns = slice(nh * NHW, (nh + 1) * NHW)
                outT = outT_p.tile([128, CC, NHW], F32, tag="outT")
                # fc1 (hi+lo DR) -> gelu(pair) -> gl fp8
                for jp in range(JC // 2):
                    pm = pm_p.tile([128, 2, NHW], F32, tag="pm")
                    for half in range(2):
                        j = 2 * jp + half
                        js = slice(j * 128, (j + 1) * 128)
                        for t in range(CC // 2):
                            nc.tensor.matmul(pm[:, half], fc1h[:, 2 * t:2 * t + 2, js],
                                             hT[:, 2 * t:2 * t + 2, ns],
                                             start=(t == 0), stop=False, perf_mode=DR)
                        for t in range(CC // 2):
                            nc.tensor.matmul(pm[:, half], fc1l[:, 2 * t:2 * t + 2, js],
                                             hT[:, 2 * t:2 * t + 2, ns],
                                             start=False, stop=(t == CC // 2 - 1),
                                             perf_mode=DR)
                    gl = gls[jp]
                    if fc1b is not None:
                        # bias is per hidden unit = per psum partition, halves differ
                        for half in range(2):
                            j = 2 * jp + half
                            nc.scalar.activation(gl[:, half], pm[:, half], AF.Gelu,
                                                 bias=fc1b[:, j:j + 1], scale=DQ_FC)
                    else:
                        nc.scalar.activation(gl.rearrange("p a b -> p (a b)"),
                                             pm.rearrange("p a b -> p (a b)"),
                                             AF.Gelu, scale=DQ_FC)
                # fc2 (hi+lo DR) per output chunk, then residual + transpose
                for mo in range(CC):
                    cs = slice(mo * 128, (mo + 1) * 128)
                    zp = pz_p.tile([128, NHW], F32, tag="pz")
                    for jp in range(JC // 2):
                        nc.tensor.matmul(zp, fc2h[:, 2 * jp:2 * jp + 2, cs], gls[jp],
                                         start=(jp == 0), stop=False, perf_mode=DR)
                    for jp in range(JC // 2):
                        nc.tensor.matmul(zp, fc2l[:, 2 * jp:2 * jp + 2, cs], gls[jp],
                                         start=False, stop=(jp == JC // 2 - 1),
                                         perf_mode=DR)
                    if fc2b is not None:
                        nc.scalar.activation(zp, zp, AF.Identity,
                                             bias=fc2b[:, mo:mo + 1], scale=DQ_FC)
                        nc.vector.tensor_tensor(outT[:, mo], f32(xT[:, mo, ns]),
                                                zp, ALU.add)
                    else:
                        nc.vector.scalar_tensor_tensor(
                            outT[:, mo], zp, DQ_FC, f32(xT[:, mo, ns]),
                            ALU.mult, ALU.add)
                # transpose out + store (4 m-chunks per half)
                outTr = outT.bitcast(F32R)
                for mq in range(NHW // 128):
                    ot = otok_p.tile([128, C], F32, tag="ot")
                    for cq in range(CC // 2):
                        ps = ps_tr3.tile([128, 2, 128], F32R, tag="tr3")
                        for half in range(2):
                            c = 2 * cq + half
                            nc.tensor.transpose(ps[:, half],
                                                outTr[:, c, mq * 128:(mq + 1) * 128],
                                                identR)
                        dst = ot[:, 2 * cq * 128:(2 * cq + 2) * 128]
                        dst = dst.rearrange("p (a b) -> p a b", a=2)
                        if (mq + cq) % 2 == 0:
                            nc.gpsimd.tensor_copy(out=dst, in_=f32(ps))
                        else:
                            nc.scalar.copy(dst, f32(ps))
                    tok0 = nh * NHW + mq * 128
                    nc.sync.dma_start(out_d[tok0:tok0 + 128, :], ot)

    nc.compile()
    return nc


_NC_CACHE = {}


def _q8(w, s=WS):
    return np.clip(w.astype(np.float64) * s, -240.0, 240.0).astype(E4NP)


def _q8_split(w, s=WS):
    ws = np.clip(w.astype(np.float64) * s, -240.0, 240.0)
    hi = ws.astype(E4NP)
    lo = np.clip(ws - hi.astype(np.float64), -240.0, 240.0).astype(E4NP)
    return hi, lo


def kernel(**inputs):
    inp = {k: np.ascontiguousarray(np.asarray(v), dtype=np.float32)
           for k, v in inputs.items()}
    flags = {
        "gb1g": not (np.all(inp["ln1_g"] == 1.0) and np.all(inp["ln1_b"] == 0.0)),
        "gb1l": not (np.all(inp["ln1l_g"] == 1.0) and np.all(inp["ln1l_b"] == 0.0)),
        "gb2": not (np.all(inp["ln2_g"] == 1.0) and np.all(inp["ln2_b"] == 0.0)),
        "bias_gproj": bool(np.any(inp["g_proj_b"] != 0.0)),
        "bias_lproj": bool(np.any(inp["l_proj_b"] != 0.0)),
        "bias_fc1": bool(np.any(inp["fc1_b"] != 0.0)),
        "bias_fc2": bool(np.any(inp["fc2_b"] != 0.0)),
    }
    key = tuple(sorted(flags.items()))
    nc = _NC_CACHE.get(key)
    if nc is None:
        nc = _build(flags)
        _NC_CACHE[key] = nc

    g_qkv = inp["g_qkv_w"]
    wv = np.zeros((GD, H * DP), np.float32)
    wv.reshape(GD, H, DP)[:, :, :D] = g_qkv[:, 2 * GD:].reshape(GD, H, D)
    fc1h, fc1l = _q8_split(inp["fc1_w"])
    fc2h, fc2l = _q8_split(inp["fc2_w"])
    weights = {
        "gqk8": _q8(g_qkv[:, :2 * GD]),
        "wv8": _q8(wv),
        "lqkv8": _q8(inp["l_qkv_w"]),
        "gp8": _q8(inp["g_proj_w"]),
        "lp8": _q8(inp["l_proj_w"]),
        "fc1h": fc1h, "fc1l": fc1l, "fc2h": fc2h, "fc2l": fc2l,
    }
    for nm, fl in (("ln1_g", "gb1g"), ("ln1_b", "gb1g"), ("ln1l_g", "gb1l"),
                   ("ln1l_b", "gb1l"), ("ln2_g", "gb2"), ("ln2_b", "gb2"),
                   ("g_proj_b", "bias_gproj"), ("l_proj_b", "bias_lproj"),
                   ("fc1_b", "bias_fc1"), ("fc2_b", "bias_fc2")):
        if flags[fl]:
            weights[nm] = inp[nm]

    x = inp["x"]
    in_maps = [dict(weights, x=np.ascontiguousarray(x[b])) for b in range(B)]
    res = run_bass_kernel_spmd(nc, in_maps, core_ids=list(range(B)))
    return np.stack([res.results[b]["out"] for b in range(B)]).astype(np.float32)
